# revision 58
# baseline (speedup 1.0000x reference)
"""GAT-Transformer (2-layer) distributed Bass kernel for 8 Trainium2 NeuronCores.

Sharding: nodes partitioned across 8 cores (5000/core, padded to 5120 = 40
blocks x 128), LPT-permuted so per-block in-degree loads balance. Edges
partitioned by destination node; segment-softmax and scatter-aggregate stay
device-local.

Key algebraic trick: the GAT aggregation is moved BEFORE the weight matmul.
Since out[dst] = sum_e attn_e * (xn[src_e] @ W) = (sum_e attn_e * xn[src_e]) @ W,
each core only shares/gathers the 128-dim LN-normalized features xn (in fp8,
plus 4 bf16 alpha_src logits: 136B/row) instead of 512-dim bf16 h rows: the
AllGather and per-edge gather shrink ~8x and the stage-A h matmul disappears.
The per-head W_h @ qf_W_h product folds into one [128,128] M_h on device, so
the GAT output never materializes: x += sum_h z_h @ M_h.

Pipeline: stage A (LN + alpha logits) for layer d+1 is emitted inside layer
d's edge loop right after each block's FF, so the chunked AllGather of layer
d+1 overlaps layer d's edge phase. After each AG chunk, a local DRAM->DRAM
DMA expands rows to a 256B stride for dma_gather (the ANT SWDGE gather; one
call per (block, src-half), int16 indices wrapped 16-wide). Per dst block:
one-hot scatter matrices S via iota+is_equal, S^T via TensorE transpose gives
per-edge alpha_dst and per-edge 1/den, attention weights scale S (one DVE op
per subtile over all 4 heads), TensorE accumulates zT = xn_g^T @ S_attn in
PSUM. Softmax max-subtraction is skipped: alpha is bounded (|a| ~ 1), exp()
cannot overflow, result matches reference well below tolerance.

LayerNorm gains/biases fold into adjacent weights on device; ff_b1 rides a
rank-1 matmul so gelu is a single bias-free [P,512] op; LN variance uses
E[x^2]-mu^2 with the free-axis sum on the ACT accumulator port.
Host-side preprocessing only reorders/partitions data (edge sort by dst then
src, LPT node permutation, index remapping, weight transposes).
"""
import math

import numpy as np
import ml_dtypes

import concourse.bacc as bacc
import concourse.mybir as mybir
import concourse.tile as tile
from concourse.bass import IndirectOffsetOnAxis
from concourse.masks import make_identity

# model dims (fixed by the problem)
D = 128         # model dim
H = 4           # heads
C = 128         # per-head channels
HC = H * C      # 512
MLP = 512
DEPTH = 2
NEG_SLOPE = 0.2
EPS_SM = 1e-16
EPS_LN = 1e-5

CORES = 8
P = 128

# scheduling knobs
TUNE = {"sb": 8, "hg": 8, "pgat": 1, "ptr": 2, "pmm": 3, "palp": 1, "pdn": 1}
import os as _os
GATHER_MODE = _os.environ.get("GATHER_MODE", "ant")  # ant | sub | block
FP8 = _os.environ.get("H_FP8", "0") == "1"
NCHUNK = int(_os.environ.get("NCHUNK", "2"))
CHUNKS = _os.environ.get("CHUNKS", "")  # e.g. "16,16,8": blocks per AG chunk

BF16 = mybir.dt.bfloat16
F32 = mybir.dt.float32
F8 = mybir.dt.float8e4
I32 = mybir.dt.int32
I16 = mybir.dt.int16
HDT = F8 if FP8 else BF16
HB = (D + 8) if FP8 else (D + H)   # gathered row cols: xn + alpha_src(bf16)
EXP = 256                          # expanded row elems for dma_gather
AF = mybir.ActivationFunctionType
ALU = mybir.AluOpType
AX = mybir.AxisListType


def _asrc_view(hg_ap):
    """bf16 view of the packed alpha_src columns of a gathered-row slice."""
    if FP8:
        return hg_ap[..., D:D + 8].bitcast(BF16)
    return hg_ap[..., D:D + H]


# ----------------------------------------------------------------------------
# device kernel builder
# ----------------------------------------------------------------------------

def build_nc(npad, ksub, rep=1, do_edges=True, do_gather=True, do_dense=True,
             do_adst=True, do_msg=True, do_coll=True):
    """Build the Bass program for one core. npad: padded local node count
    (multiple of 128); ksub: 128-edge subtiles per 128-node block — an int
    (indirect-DMA per-subtile gather) or a (ksub_lo, ksub_hi) tuple
    (dma_gather with the node range split for int16 indices).
    rep>1 repeats the whole network body (timing only). do_* flags disable
    pipeline stages for perf bisection (results wrong when False)."""
    if isinstance(ksub, tuple):
        ksub_lo, ksub_hi = ksub
        ksub = ksub_lo + ksub_hi
        ant_gather = True
    else:
        ant_gather = False
    nblk = npad // P
    nsub = nblk * ksub
    ng = CORES * npad  # global padded nodes
    split = (CORES // 2) * npad  # lo/hi node-range split (int16 idx limit)
    R = DEPTH * rep
    if not ant_gather:
        cbs = [nblk]
    elif CHUNKS:
        cbs = [int(t) for t in CHUNKS.split(",")]
    elif nblk == 40:
        cbs = [14, 14, 12]
    else:
        nch = min(NCHUNK, nblk)
        cbs = [nblk // nch] * nch
        cbs[-1] += nblk - sum(cbs)
    assert sum(cbs) == nblk
    nck = len(cbs)
    cb0 = [sum(cbs[:i]) for i in range(nck)]  # chunk start block

    from contextlib import ExitStack
    nc = bacc.Bacc(None, target_bir_lowering=False, debug=False)
    with tile.TileContext(nc) as tc, ExitStack() as es:
        dram = es.enter_context(tc.tile_pool(name="dram", bufs=1, space="DRAM"))
        const = es.enter_context(tc.tile_pool(name="const", bufs=1))
        wpool = es.enter_context(tc.tile_pool(name="wpool", bufs=1))
        sb = es.enter_context(tc.tile_pool(name="sb", bufs=TUNE["sb"]))
        hg = es.enter_context(tc.tile_pool(name="hg", bufs=TUNE["hg"]))
        pgat = es.enter_context(tc.tile_pool(name="pgat", bufs=TUNE["pgat"], space="PSUM"))
        palpha = es.enter_context(tc.tile_pool(name="palpha", bufs=TUNE["palp"], space="PSUM"))
        pdn = es.enter_context(tc.tile_pool(name="pdn", bufs=TUNE["pdn"], space="PSUM"))
        ptr = es.enter_context(tc.tile_pool(name="ptr", bufs=TUNE["ptr"], space="PSUM"))
        pmm = es.enter_context(tc.tile_pool(name="pmm", bufs=TUNE["pmm"], space="PSUM"))
        p512 = pmm
        p128 = pmm

        # ---- I/O ------------------------------------------------------------
        def einp(name, shape, dtype=F32):
            return dram.tile(shape, dtype, kind="ExternalInput", name=name,
                             uniquify=False)

        x_in = einp("x", [npad, D])
        src_idx = einp("src_idx", [P, nsub], I32)      # global padded row
        gidx_in = einp("gidx", [P, nsub * 8], I16)     # wrapped idx, dma_gather
        dst_loc = einp("dst_loc", [P, nsub])           # dst within block, 255=pad
        ea_in = einp("ea", [P, nsub])                  # edge_attr, sorted order
        w_gat = einp("gat_W", [DEPTH, D, HC])
        a_srcT = einp("att_srcT", [DEPTH, C, H])
        a_dstT = einp("att_dstT", [DEPTH, C, H])
        ew_T = einp("edge_WT", [DEPTH, C, H])
        ae_T = einp("att_edgeT", [DEPTH, C, H])
        gb_T = einp("gat_biasT", [DEPTH, C, H])
        w_qf = einp("qf_W", [DEPTH, HC, D])
        b_qf = einp("qf_b", [DEPTH, D])
        ln1gT = einp("ln1_gT", [DEPTH, D, 1])
        ln1bT = einp("ln1_bT", [DEPTH, D, 1])
        ln2gT = einp("ln2_gT", [DEPTH, D, 1])
        ln2bT = einp("ln2_bT", [DEPTH, D, 1])
        w_f1 = einp("ff_W1", [DEPTH, D, MLP])
        b_f1r = einp("ff_b1r", [DEPTH, 1, MLP])
        w_f2 = einp("ff_W2", [DEPTH, MLP, D])
        b_f2 = einp("ff_b2", [DEPTH, D])

        x_out = dram.tile([npad, D], F32, kind="ExternalOutput", name="x_out",
                          uniquify=False)

        h_locs = [dram.tile([npad, HB], HDT, name=f"h_loc{d}")
                  for d in range(R)]
        if ant_gather:
            h_fullc = [[dram.tile([CORES * cbs[c] * P, HB], HDT,
                                  addr_space="Shared" if do_coll else "Local",
                                  name=f"h_full{d}_{c}")
                        for c in range(nck)] for d in range(R)]
            h_exps = [dram.tile([ng, EXP], HDT, name=f"h_exp{d}")
                      for d in range(R)]
        else:
            h_fulls = [dram.tile([ng, HB], HDT,
                                 addr_space="Shared" if do_coll else "Local",
                                 name=f"h_full{d}")
                       for d in range(R)]

        # ---- static constants ----------------------------------------------
        iota_i = const.tile([P, P], I32)
        nc.gpsimd.iota(iota_i[:], pattern=[[1, P]], base=0, channel_multiplier=0)
        iota_bf = const.tile([P, P], BF16)
        nc.vector.tensor_copy(out=iota_bf[:], in_=iota_i[:])
        ident_bf = const.tile([P, P], BF16)
        make_identity(nc, ident_bf)
        ones1_bf = const.tile([1, P], BF16)
        nc.vector.memset(ones1_bf[:], 1.0)
        one11_bf = const.tile([1, 1], BF16)
        nc.vector.memset(one11_bf[:], 1.0)
        epsln = const.tile([P, 1], F32)
        nc.vector.memset(epsln[:], EPS_LN)

        x_sb = const.tile([P, nblk, D], F32)
        nc.sync.dma_start(out=x_sb[:], in_=x_in[:].rearrange("(b p) f -> p b f", p=P))
        if ant_gather:
            gidx_sb = const.tile([P, nsub * 8], I16)
            nc.sync.dma_start(out=gidx_sb[:], in_=gidx_in[:])
        else:
            srcx_sb = const.tile([P, nsub], I32)
            nc.sync.dma_start(out=srcx_sb[:], in_=src_idx[:])
        dstl_bf = const.tile([P, nsub], BF16)
        nc.gpsimd.dma_start(out=dstl_bf[:], in_=dst_loc[:])
        dstl_f = const.tile([P, nsub], F32)
        nc.sync.dma_start(out=dstl_f[:], in_=dst_loc[:])
        ea_sb = const.tile([P, nsub], F32)
        nc.sync.dma_start(out=ea_sb[:], in_=ea_in[:])
        adst_par = [const.tile([P, nblk, H], BF16, name=f"adst{par}")
                    for par in range(DEPTH)]

        # ---------------- per-layer weight prep (both layers upfront) -------
        ws = []
        for d in range(DEPTH):
            W = {}

            def wt(shape, dtype, nm):
                return wpool.tile(shape, dtype, name=f"{nm}{d}")

            w_bf = wt([D, HC], BF16, "w_bf")
            nc.gpsimd.dma_start(out=w_bf[:], in_=w_gat[d])   # cast f32->bf16
            qfw_bf = wt([P, H, D], BF16, "qfw_bf")
            nc.gpsimd.dma_start(out=qfw_bf[:],
                                in_=w_qf[d].rearrange("(c p) n -> p c n", p=P))
            f1w_bf = wt([D, MLP], BF16, "f1w_bf")
            nc.gpsimd.dma_start(out=f1w_bf[:], in_=w_f1[d])
            f2w_bf = wt([P, H, D], BF16, "f2w_bf")
            nc.gpsimd.dma_start(out=f2w_bf[:],
                                in_=w_f2[d].rearrange("(c p) n -> p c n", p=P))
            att_bf = wt([C, 4, H], BF16, "att_bf")  # src,dst,ew,ae cols
            nc.gpsimd.dma_start(out=att_bf[:, 0, :], in_=a_srcT[d])
            nc.gpsimd.dma_start(out=att_bf[:, 1, :], in_=a_dstT[d])
            nc.gpsimd.dma_start(out=att_bf[:, 2, :], in_=ew_T[d])
            nc.gpsimd.dma_start(out=att_bf[:, 3, :], in_=ae_T[d])
            gbT_bf = wt([C, H], BF16, "gbT_bf")
            nc.gpsimd.dma_start(out=gbT_bf[:], in_=gb_T[d])
            qfb_bf = wt([1, D], BF16, "qfb_bf")
            nc.gpsimd.dma_start(out=qfb_bf[:], in_=b_qf[d:d + 1, :])
            f2b_bf = wt([1, D], BF16, "f2b_bf")
            nc.gpsimd.dma_start(out=f2b_bf[:], in_=b_f2[d:d + 1, :])
            f1b_bf = wt([1, MLP], BF16, "f1b_bf")
            nc.gpsimd.dma_start(out=f1b_bf[:], in_=b_f1r[d])
            g1c = wt([D, 1], F32, "g1c")
            nc.sync.dma_start(out=g1c[:], in_=ln1gT[d])
            b1c = wt([D, 1], BF16, "b1c")
            nc.gpsimd.dma_start(out=b1c[:], in_=ln1bT[d])
            g2c = wt([D, 1], F32, "g2c")
            nc.sync.dma_start(out=g2c[:], in_=ln2gT[d])
            b2c = wt([D, 1], BF16, "b2c")
            nc.gpsimd.dma_start(out=b2c[:], in_=ln2bT[d])

            # fold LN1 gain into W (h = xn @ w_s + b1^T W)
            w_s = wt([D, HC], BF16, "w_s")
            nc.vector.tensor_scalar_mul(out=w_s[:], in0=w_bf[:], scalar1=g1c[:, :1])

            # W transposed per head (unscaled, for Wa) and scaled (for M)
            wT_bf = wt([C, H, D], BF16, "wT_bf")
            wsT_bf = wt([C, H, D], BF16, "wsT_bf")
            for h in range(H):
                pst = ptr.tile([P, P], BF16, space="PSUM", name="pst")
                nc.tensor.transpose(out=pst[:], in_=w_bf[:, h * C:(h + 1) * C],
                                    identity=ident_bf[:])
                nc.any.tensor_copy(out=wT_bf[:, h, :], in_=pst[:])
                pst2 = ptr.tile([P, P], BF16, space="PSUM", name="pst")
                nc.tensor.transpose(out=pst2[:], in_=w_s[:, h * C:(h + 1) * C],
                                    identity=ident_bf[:])
                nc.any.tensor_copy(out=wsT_bf[:, h, :], in_=pst2[:])
            # Wa[:, 0:4] = W_h @ a_src_h ; [:, 4:8] = W_h @ a_dst_h
            pwa = p128.tile([D, 2 * H], F32, space="PSUM", name="p128t", tag="pmmt")
            for h in range(H):
                nc.tensor.matmul(out=pwa[:, h:h + 1], lhsT=wT_bf[:, h, :],
                                 rhs=att_bf[:, 0, h:h + 1], start=True, stop=True)
                nc.tensor.matmul(out=pwa[:, H + h:H + h + 1], lhsT=wT_bf[:, h, :],
                                 rhs=att_bf[:, 1, h:h + 1], start=True, stop=True)
            wa_bf = wt([D, 2 * H], BF16, "wa_bf")
            nc.any.tensor_copy(out=wa_bf[:], in_=pwa[:])
            # fold LN1 gain into Wa; LN1 bias becomes rank-1 rows
            wa_s = wt([D, 2 * H], BF16, "wa_s")
            nc.vector.tensor_scalar_mul(out=wa_s[:], in0=wa_bf[:], scalar1=g1c[:, :1])
            pbwa = p128.tile([1, 2 * H], F32, space="PSUM", name="pbwa", tag="pmmt")
            nc.tensor.matmul(out=pbwa[:], lhsT=b1c[:], rhs=wa_bf[:],
                             start=True, stop=True)
            bwa_row = wt([1, 2 * H], BF16, "bwa_row")
            nc.any.tensor_copy(out=bwa_row[:], in_=pbwa[:])
            # per-head b1^T @ W_h as columns (for the qf-folded bias)
            pbwc = p128.tile([C, H], F32, space="PSUM", name="pbwc", tag="pmmt")
            for h in range(H):
                nc.tensor.matmul(out=pbwc[:, h:h + 1],
                                 lhsT=w_bf[:, h * C:(h + 1) * C], rhs=b1c[:],
                                 start=True, stop=True)
            bwc_bf = wt([C, H], BF16, "bwc_bf")
            nc.any.tensor_copy(out=bwc_bf[:], in_=pbwc[:])
            # fold LN2 gain into ff_W1; LN2 bias + ff_b1 -> rank-1 row
            f1w_s = wt([D, MLP], BF16, "f1w_s")
            nc.vector.tensor_scalar_mul(out=f1w_s[:], in0=f1w_bf[:], scalar1=g2c[:, :1])
            pbw1 = p512.tile([1, MLP], F32, space="PSUM", name="pbw1", tag="pmmt")
            nc.tensor.matmul(out=pbw1[:], lhsT=b2c[:], rhs=f1w_bf[:],
                             start=True, stop=True)
            bw1_row = wt([1, MLP], BF16, "bw1_row")
            nc.vector.tensor_tensor(out=bw1_row[:], in0=pbw1[:], in1=f1b_bf[:],
                                    op=ALU.add)

            # M_h = (diag(g1) W)_h @ qf_W_h   [128, 128] per head
            m_bf = wt([P, H, D], BF16, "m_bf")
            for h in range(H):
                pM = p128.tile([P, D], F32, space="PSUM", name="p128m", tag="pmmt")
                nc.tensor.matmul(out=pM[:], lhsT=wsT_bf[:, h, :],
                                 rhs=qfw_bf[:, h, :], start=True, stop=True)
                nc.any.tensor_copy(out=m_bf[:, h, :], in_=pM[:])

            # we_dot[h] = <edge_W_h, a_edge_h>, replicated [P, ksub*H]
            pwe = p128.tile([1, H], F32, space="PSUM", name="p128t2", tag="pmmt")
            for h in range(H):
                nc.tensor.matmul(out=pwe[:, h:h + 1], lhsT=att_bf[:, 2, h:h + 1],
                                 rhs=att_bf[:, 3, h:h + 1], start=True, stop=True)
            wd_row = wt([1, H], BF16, "wd_row")
            nc.any.tensor_copy(out=wd_row[:], in_=pwe[:])
            wd_rep = wt([1, ksub, H], BF16, "wd_rep")
            nc.vector.tensor_copy(
                out=wd_rep[:],
                in_=wd_row[:].unsqueeze(1).to_broadcast([1, ksub, H]))
            pwr = p512.tile([P, ksub * H], F32, space="PSUM", name="p512t", tag="pmmt")
            nc.tensor.matmul(out=pwr[:], lhsT=ones1_bf[:],
                             rhs=wd_rep[:].rearrange("a k h -> a (k h)"),
                             start=True, stop=True)
            wedot = wt([P, ksub * H], F32, "wedot")
            nc.any.tensor_copy(out=wedot[:], in_=pwr[:])

            # qfb_eff = gat_bias @ qf_W + qf_b   -> [1, D]
            pqb = p128.tile([1, D], F32, space="PSUM", name="p128t3", tag="pmmt")
            for cch in range(H):
                nc.tensor.matmul(out=pqb[:], lhsT=gbT_bf[:, cch:cch + 1],
                                 rhs=qfw_bf[:, cch, :], start=(cch == 0), stop=False)
            nc.tensor.matmul(out=pqb[:], lhsT=one11_bf[:], rhs=qfb_bf[:],
                             start=False, stop=True)
            qfbe = wt([1, D], BF16, "qfbe")
            nc.any.tensor_copy(out=qfbe[:], in_=pqb[:])
            # (b1^T W) @ qf_W broadcast to all partitions; applied per dst
            # gated by [deg>0] since sum_e attn = 0 for isolated nodes
            pqw = p128.tile([1, D], F32, space="PSUM", name="p128t4", tag="pmmt")
            for cch in range(H):
                nc.tensor.matmul(out=pqw[:], lhsT=bwc_bf[:, cch:cch + 1],
                                 rhs=qfw_bf[:, cch, :], start=(cch == 0),
                                 stop=(cch == H - 1))
            qbw_row = wt([1, D], BF16, "qbw_row")
            nc.any.tensor_copy(out=qbw_row[:], in_=pqw[:])
            pqwb = p128.tile([P, D], F32, space="PSUM", name="p128t5", tag="pmmt")
            nc.tensor.matmul(out=pqwb[:], lhsT=ones1_bf[:], rhs=qbw_row[:],
                             start=True, stop=True)
            qbw_bc = wt([P, D], BF16, "qbw_bc")
            nc.any.tensor_copy(out=qbw_bc[:], in_=pqwb[:])

            W.update(w_s=w_s, wa_s=wa_s, bwa_row=bwa_row, f1w_s=f1w_s,
                     bw1_row=bw1_row, f2w_bf=f2w_bf, f2b_bf=f2b_bf, m_bf=m_bf,
                     wedot=wedot, qfbe=qfbe, qbw_bc=qbw_bc)
            ws.append(W)

        # ---------------- batched layer-norm statistics ----------------------
        # One Sqrt per stage (vs per block): activation-table reloads are
        # ~1.3us each and sqrt/exp/gelu live in different table sets, so
        # per-block LNs force constant reloads. Square/Copy are in EVERY set
        # and never force a reload.
        def ln_stats(name, b0=0, nb=None):
            """LN stats for blocks [b0, b0+nb); returns [P, nb] mu/rstd
            indexed by b - b0."""
            nb = nblk if nb is None else nb
            xv = x_sb[:, b0:b0 + nb, :]
            sx = sb.tile([P, nb], F32, name=f"sx{name}", bufs=2)
            nc.vector.tensor_reduce(out=sx[:].unsqueeze(2),
                                    in_=xv, axis=AX.X, op=ALU.add)
            sx2 = sb.tile([P, nb], F32, name=f"sx2{name}", bufs=2)
            for b in range(nb):
                sq = sb.tile([P, D], F32, name=f"sq{name}")
                nc.scalar.activation(out=sq[:], in_=x_sb[:, b0 + b, :],
                                     func=AF.Square,
                                     accum_out=sx2[:, b:b + 1])
            mu = sb.tile([P, nb], F32, name=f"mu{name}", bufs=2)
            nc.vector.tensor_scalar_mul(out=mu[:], in0=sx[:], scalar1=1.0 / D)
            mu2 = sb.tile([P, nb], F32, name=f"mu2{name}")
            nc.vector.tensor_tensor(out=mu2[:], in0=mu[:], in1=mu[:], op=ALU.mult)
            var = sb.tile([P, nb], F32, name=f"var{name}")
            nc.vector.tensor_scalar(out=var[:], in0=sx2[:], scalar1=1.0 / D,
                                    scalar2=None, op0=ALU.mult)
            nc.vector.tensor_tensor(out=var[:], in0=var[:], in1=mu2[:],
                                    op=ALU.subtract)
            std = sb.tile([P, nb], F32, name=f"std{name}")
            nc.scalar.activation(out=std[:], in_=var[:], func=AF.Sqrt,
                                 bias=epsln[:, :1])
            rstd = sb.tile([P, nb], F32, name=f"rstd{name}", bufs=2)
            nc.vector.reciprocal(out=rstd[:], in_=std[:])
            return mu, rstd, b0

        def ln_apply(b, stats, name, xn_out=None):
            """xn = (x - mu)*rstd for one block + transposed copy."""
            mu, rstd, b0 = stats
            if xn_out is None:
                xn_out = sb.tile([P, D], BF16, name=f"xn{name}")
            nc.vector.tensor_scalar(out=xn_out[:], in0=x_sb[:, b, :],
                                    scalar1=mu[:, b - b0:b - b0 + 1],
                                    scalar2=rstd[:, b - b0:b - b0 + 1],
                                    op0=ALU.subtract, op1=ALU.mult)
            pst = ptr.tile([P, P], BF16, space="PSUM", name="pst")
            nc.tensor.transpose(out=pst[:], in_=xn_out[:], identity=ident_bf[:])
            xnT = sb.tile([P, P], BF16, name=f"xnT{name}")
            nc.any.tensor_copy(out=xnT[:], in_=pst[:])
            return xnT, xn_out

        # ---------------- stage A: xn + alpha logits for one block ----------
        def stage_a(rd, b, stats):
            Wd = ws[rd % DEPTH]
            h_sb = sb.tile([P, HB], HDT, name="h_sb")
            if FP8:
                xnT, xn_bf = ln_apply(b, stats, "A")
                nc.vector.tensor_copy(out=h_sb[:, 0:D], in_=xn_bf[:])
            else:
                xnT, _ = ln_apply(b, stats, "A", xn_out=h_sb[:, 0:D])
            pa8 = p128.tile([P, 2 * H], F32, space="PSUM", name="pa8", tag="pmmt")
            nc.tensor.matmul(out=pa8[:], lhsT=xnT[:], rhs=Wd["wa_s"][:],
                             start=True, stop=False)
            nc.tensor.matmul(out=pa8[:], lhsT=ones1_bf[:], rhs=Wd["bwa_row"][:],
                             start=False, stop=True)
            nc.any.tensor_copy(out=_asrc_view(h_sb)[:, :], in_=pa8[:, 0:H])
            nc.any.tensor_copy(out=adst_par[rd % DEPTH][:, b, :],
                               in_=pa8[:, H:2 * H])
            nc.sync.dma_start(out=h_locs[rd][b * P:(b + 1) * P, :], in_=h_sb[:])

        # ---------------- AG chunk + row expand ------------------------------
        def emit_ag(rd, ck):
            h_loc = h_locs[rd]
            if ant_gather:
                hf = h_fullc[rd][ck]
                r0, r1 = cb0[ck] * P, (cb0[ck] + cbs[ck]) * P
                if do_coll:
                    nc.gpsimd.collective_compute(
                        "AllGather", ALU.bypass,
                        replica_groups=[list(range(CORES))],
                        ins=[h_loc[r0:r1, :].opt()],
                        outs=[hf[:].opt()],
                    )
                else:
                    nc.sync.dma_start(out=hf[0:r1 - r0, :], in_=h_loc[r0:r1, :])
                exp_view = h_exps[rd][:].rearrange("(r n) f -> r n f", r=CORES)
                nc.sync.dma_start(
                    out=exp_view[:, r0:r1, 0:HB],
                    in_=hf[:].rearrange("(r n) f -> r n f", r=CORES))
            else:
                if do_coll:
                    nc.gpsimd.collective_compute(
                        "AllGather", ALU.bypass,
                        replica_groups=[list(range(CORES))],
                        ins=[h_loc[:].opt()],
                        outs=[h_fulls[rd][:].opt()],
                    )
                else:
                    nc.sync.dma_start(out=h_fulls[rd][0:npad, :], in_=h_loc[:])

        # ---------------- tail: ff for one block -----------------------------
        def emit_ff(rd, b, stats):
            if not do_dense:
                return
            Wd = ws[rd % DEPTH]
            xn2T, _ = ln_apply(b, stats, "B")
            pa1 = p512.tile([P, MLP], F32, space="PSUM", name="pa1", tag="pmmt")
            a1T = sb.tile([P, H, P], BF16, name="a1T")
            for cch in range(MLP // P):
                nc.tensor.matmul(out=pa1[:, cch * P:(cch + 1) * P],
                                 lhsT=Wd["f1w_s"][:, cch * P:(cch + 1) * P],
                                 rhs=xn2T[:], start=True, stop=False)
                nc.tensor.matmul(out=pa1[:, cch * P:(cch + 1) * P],
                                 lhsT=Wd["bw1_row"][:, cch * P:(cch + 1) * P],
                                 rhs=ones1_bf[:], start=False, stop=True)
            nc.scalar.activation(
                out=a1T[:].rearrange("p h q -> p (h q)"), in_=pa1[:],
                func=AF.Gelu_apprx_tanh)
            pf2 = p128.tile([P, D], F32, space="PSUM", name="pf2", tag="pmmt")
            for cch in range(MLP // P):
                nc.tensor.matmul(out=pf2[:], lhsT=a1T[:, cch, :],
                                 rhs=Wd["f2w_bf"][:, cch, :],
                                 start=(cch == 0), stop=False)
            nc.tensor.matmul(out=pf2[:], lhsT=ones1_bf[:], rhs=Wd["f2b_bf"][:],
                             start=False, stop=True)
            nc.vector.tensor_add(out=x_sb[:, b, :], in0=x_sb[:, b, :], in1=pf2[:])

        # ---------------- edge phase + qf for one block ----------------------
        def edge_block(rd, b):
            Wd = ws[rd % DEPTH]
            adst_all = adst_par[rd % DEPTH]
            hgat = hg.tile([P, ksub, EXP if ant_gather else HB], HDT,
                           name="hgat")
            if not do_gather:
                nc.sync.dma_start(
                    out=hgat[:, :, 0:HB],
                    in_=(h_exps[rd] if ant_gather else h_fulls[rd])
                        [0:ksub * P, 0:HB].rearrange("(k p) f -> p k f", p=P))
            elif ant_gather:
                h_exp = h_exps[rd]
                gcols = ksub * P // 16
                g0 = b * gcols
                for (kq0, kq1, base) in ((0, ksub_lo, 0),
                                         (ksub_lo, ksub, split)):
                    kk = kq1 - kq0
                    if kk == 0:
                        continue
                    nc.gpsimd.dma_gather(
                        out_ap=hgat[:, kq0:kq1, :],
                        in_ap=h_exp[base:base + split, :],
                        idxs_ap=gidx_sb[:, g0 + kq0 * 8:g0 + kq1 * 8],
                        num_idxs=kk * P, num_idxs_reg=kk * P,
                        elem_size=EXP)
            elif GATHER_MODE == "block":
                nc.gpsimd.indirect_dma_start(
                    out=hgat[:], out_offset=None, in_=h_fulls[rd][:],
                    in_offset=IndirectOffsetOnAxis(
                        ap=srcx_sb[:, b * ksub:(b + 1) * ksub], axis=0))
            else:
                for k in range(ksub):
                    s = b * ksub + k
                    nc.gpsimd.indirect_dma_start(
                        out=hgat[:, k, :], out_offset=None, in_=h_fulls[rd][:],
                        in_offset=IndirectOffsetOnAxis(
                            ap=srcx_sb[:, s:s + 1], axis=0))
            s_all = sb.tile([P, ksub, P], BF16, name="s_all", bufs=2)
            nc.gpsimd.tensor_tensor(
                out=s_all[:],
                in0=iota_bf[:].unsqueeze(1).to_broadcast([P, ksub, P]),
                in1=dstl_bf[:, b * ksub:(b + 1) * ksub].unsqueeze(2)
                    .to_broadcast([P, ksub, P]),
                op=ALU.is_equal)
            sT_all = sb.tile([P, ksub, P], BF16, name="sT_all", bufs=2)
            palp = palpha.tile([P, ksub * H], F32, space="PSUM", name="palp",
                               tag="palp")
            for k0 in range(0, ksub, 2):
                kp = min(2, ksub - k0)
                pst = ptr.tile([P, 2, P], BF16, space="PSUM", name="pst")
                for j in range(kp):
                    nc.tensor.transpose(out=pst[:, j, :],
                                        in_=s_all[:, k0 + j, :],
                                        identity=ident_bf[:])
                nc.scalar.activation(out=sT_all[:, k0:k0 + kp, :],
                                     in_=pst[:, 0:kp, :], func=AF.Copy)
                if do_adst:
                    for j in range(kp):
                        k = k0 + j
                        nc.tensor.matmul(out=palp[:, k * H:(k + 1) * H],
                                         lhsT=sT_all[:, k, :],
                                         rhs=adst_all[:, b, :],
                                         start=True, stop=True)
            # alpha = asrc + adst + ea*wedot; lrelu; exp (whole block)
            ex_bf = sb.tile([P, ksub * H], BF16, name="ex_bf")
            al1 = sb.tile([P, ksub * H], F32, name="al1")
            if do_adst:
                nc.vector.tensor_tensor(
                    out=al1[:].rearrange("p (k h) -> p k h", h=H),
                    in0=palp[:].rearrange("p (k h) -> p k h", h=H),
                    in1=_asrc_view(hgat)[:, :, :], op=ALU.add)
            else:
                nc.vector.tensor_copy(
                    out=al1[:].rearrange("p (k h) -> p k h", h=H),
                    in_=_asrc_view(hgat)[:, :, :])
            aef = sb.tile([P, ksub, H], F32, name="aef")
            nc.vector.tensor_tensor(
                out=aef[:],
                in0=Wd["wedot"][:].rearrange("p (k h) -> p k h", h=H),
                in1=ea_sb[:, b * ksub:(b + 1) * ksub].unsqueeze(2)
                    .to_broadcast([P, ksub, H]),
                op=ALU.mult)
            al2 = sb.tile([P, ksub * H], F32, name="al2")
            nc.vector.tensor_tensor(
                out=al2[:], in0=al1[:],
                in1=aef[:].rearrange("p k h -> p (k h)"), op=ALU.add)
            lr = sb.tile([P, ksub * H], F32, name="lr")
            nc.vector.scalar_tensor_tensor(
                out=lr[:], in0=al2[:], scalar=NEG_SLOPE, in1=al2[:],
                op0=ALU.mult, op1=ALU.max)
            nc.scalar.activation(out=ex_bf[:], in_=lr[:], func=AF.Exp)

            # denominators: den = S^T @ ex  (per head)
            pd = pdn.tile([P, H], F32, space="PSUM", name="pd")
            for k in range(ksub):
                nc.tensor.matmul(out=pd[:], lhsT=s_all[:, k, :],
                                 rhs=ex_bf[:, k * H:(k + 1) * H],
                                 start=(k == 0), stop=(k == ksub - 1))
            den = sb.tile([P, H], F32, name="den")
            nc.vector.tensor_scalar_add(out=den[:], in0=pd[:], scalar1=EPS_SM)
            ind = sb.tile([P, 1], F32, name="ind")
            nc.vector.tensor_scalar(out=ind[:], in0=pd[:, 0:1],
                                    scalar1=1e30, scalar2=1.0,
                                    op0=ALU.mult, op1=ALU.min)
            rec = sb.tile([P, H], F32, name="rec")
            nc.vector.reciprocal(out=rec[:], in_=den[:])
            rec_bf = sb.tile([P, H], BF16, name="rec_bf")
            nc.vector.tensor_copy(out=rec_bf[:], in_=rec[:])
            # per-edge 1/den via S^T gather-matmul, then attn = ex * rec_e
            prec = palpha.tile([P, ksub * H], F32, space="PSUM", name="prec",
                               tag="palp")
            for k in range(ksub):
                nc.tensor.matmul(out=prec[:, k * H:(k + 1) * H],
                                 lhsT=sT_all[:, k, :], rhs=rec_bf[:],
                                 start=True, stop=True)
            attn = sb.tile([P, ksub * H], F32, name="attn")
            nc.vector.tensor_tensor(out=attn[:], in0=ex_bf[:], in1=prec[:],
                                    op=ALU.mult)

            # zT_h = xn_g^T @ (S * attn_h), accumulated over subtiles.
            # S*attn built directly: (iota == dstl) * attn, one fused
            # tensor_scalar per (subtile, head) -> DVE 4x mode eligible.
            pz = pgat.tile([P, H * P], F32, space="PSUM", name="pz")
            for k in range(ksub):
                sat = sb.tile([P, H, P], BF16, name="sat", bufs=4)
                for h in range(H):
                    nc.vector.tensor_scalar(
                        out=sat[:, h, :], in0=iota_bf[:],
                        scalar1=dstl_f[:, b * ksub + k:b * ksub + k + 1],
                        scalar2=attn[:, k * H + h:k * H + h + 1],
                        op0=ALU.is_equal, op1=ALU.mult)
                rhs = (sat[:].rearrange("p h q -> p (h q)") if do_msg else
                       s_all[:, k, :].unsqueeze(1).to_broadcast([P, H, P])
                       .rearrange("p h q -> p (h q)"))
                nc.tensor.matmul(out=pz[:], lhsT=hgat[:, k, 0:D], rhs=rhs,
                                 start=(k == 0), stop=(k == ksub - 1))
            zt = sb.tile([P, H * P], BF16, name="zt")
            nc.scalar.activation(out=zt[:], in_=pz[:], func=AF.Copy)

            if do_dense:
                # qf: pxs[b] = sum_h z_h @ M_h + qfb_eff + [deg>0]*(b1 W qf_W)
                # staged to SBUF; added to x in one batched op per layer
                px = p128.tile([P, D], F32, space="PSUM", name="px", tag="pmmt")
                for h in range(H):
                    nc.tensor.matmul(out=px[:], lhsT=zt[:, h * P:(h + 1) * P],
                                     rhs=Wd["m_bf"][:, h, :],
                                     start=(h == 0), stop=False)
                nc.tensor.matmul(out=px[:], lhsT=ones1_bf[:], rhs=Wd["qfbe"][:],
                                 start=False, stop=True)
                gbw = sb.tile([P, D], BF16, name="gbw")
                nc.any.tensor_scalar_mul(out=gbw[:], in0=Wd["qbw_bc"][:],
                                         scalar1=ind[:, :1])
                nc.vector.tensor_tensor(out=pxs_all[:, b, :], in0=px[:],
                                        in1=gbw[:], op=ALU.add)

        # ---------------- main pipeline --------------------------------------
        # Per layer: E-pass (edges; ACT runs exp only), one batched x += pxs,
        # F-pass (FF; ACT runs gelu only), A-pass (next layer's stage A).
        # Each LN stage shares one batched Sqrt.
        pxs_all = const.tile([P, nblk, D], BF16, name="pxs_all")
        st0 = ln_stats("A")
        for c in range(nck):
            for b in range(cb0[c], cb0[c] + cbs[c]):
                stage_a(0, b, st0)
            emit_ag(0, c)
        # Per chunk of blocks: edge phase -> qf add -> FF -> next stage A ->
        # AG chunk. The AG chunks of layer rd+1 then overlap the remaining
        # edge-phase chunks of layer rd.
        for rd in range(R):
            for c in range(nck):
                b0, bpc = cb0[c], cbs[c]
                if do_edges:
                    for b in range(b0, b0 + bpc):
                        edge_block(rd, b)
                    if do_dense:
                        nc.vector.tensor_tensor(
                            out=x_sb[:, b0:b0 + bpc, :],
                            in0=x_sb[:, b0:b0 + bpc, :],
                            in1=pxs_all[:, b0:b0 + bpc, :], op=ALU.add)
                if do_dense:
                    stf = ln_stats("B", b0, bpc)
                    for b in range(b0, b0 + bpc):
                        emit_ff(rd, b, stf)
                if rd + 1 < R:
                    sta = ln_stats("A", b0, bpc)
                    for b in range(b0, b0 + bpc):
                        stage_a(rd + 1, b, sta)
                    emit_ag(rd + 1, c)

        nc.sync.dma_start(out=x_out[:].rearrange("(b p) f -> p b f", p=P),
                          in_=x_sb[:])
    nc.finalize()
    return nc


# ----------------------------------------------------------------------------
# host-side sharding / preprocessing
# ----------------------------------------------------------------------------

def preprocess(x, edge_index, edge_attr):
    n = x.shape[0]
    e = edge_index.shape[1]
    assert n % CORES == 0
    nloc = n // CORES
    npad = ((nloc + P - 1) // P) * P
    nblk = npad // P

    src = np.asarray(edge_index[0], dtype=np.int64)
    dst = np.asarray(edge_index[1], dtype=np.int64)
    dev = dst // nloc

    split = (CORES // 2) * npad
    ant = GATHER_MODE == "ant"
    lo_edge = src < (CORES // 2) * nloc  # src on cores 0..3 -> row < split

    # LPT-pack local nodes into blocks so per-block in-degree sums balance.
    # For the ant (dma_gather) mode, balance the lo/hi src-range loads
    # jointly since each is padded to its own subtile count.
    # pos[c, i] = padded row of local node i of core c; order[c, r] = local
    # node at padded row r (-1 = hole).
    pos = np.empty((CORES, nloc), dtype=np.int64)
    order = np.full((CORES, npad), -1, dtype=np.int64)
    for c in range(CORES):
        sel_c = dev == c
        dst_c = dst[sel_c] - c * nloc
        deg_lo = np.bincount(dst_c[lo_edge[sel_c]], minlength=nloc)
        deg_hi = np.bincount(dst_c[~lo_edge[sel_c]], minlength=nloc)
        degc = deg_lo + deg_hi
        byd = np.argsort(-degc, kind="stable")
        load_lo = np.zeros(nblk, dtype=np.int64)
        load_hi = np.zeros(nblk, dtype=np.int64)
        fill = np.zeros(nblk, dtype=np.int64)
        for i in byd:
            cand = np.nonzero(fill < P)[0]
            if ant:
                score = np.maximum(load_lo[cand] + deg_lo[i],
                                   load_hi[cand] + deg_hi[i])
            else:
                score = load_lo[cand] + deg_lo[i] + load_hi[cand] + deg_hi[i]
            bsel = cand[np.argmin(score)]
            pos[c, i] = bsel * P + fill[bsel]
            order[c, bsel * P + fill[bsel]] = i
            load_lo[bsel] += deg_lo[i]
            load_hi[bsel] += deg_hi[i]
            fill[bsel] += 1

    # remap to padded (permuted) ids
    src_p = (src // nloc) * npad + pos[src // nloc, src % nloc]
    dst_p = (dst // nloc) * npad + pos[dev, dst % nloc]

    ea = np.asarray(edge_attr, dtype=np.float32).reshape(-1)

    per_dev = []
    klo_max, khi_max, ksub1 = 1, 0, 1
    for dcore in range(CORES):
        sel = np.nonzero(dev == dcore)[0]
        eorder = np.argsort(dst_p[sel], kind="stable")
        sel = sel[eorder]
        dloc = dst_p[sel] - dcore * npad          # [0, npad)
        blk = dloc // P
        cnt = np.bincount(blk, minlength=nblk)
        cnt_lo = np.bincount(blk[lo_edge[sel]], minlength=nblk)
        klo_max = max(klo_max, int(math.ceil(cnt_lo.max() / P)))
        khi_max = max(khi_max,
                      int(math.ceil((cnt - cnt_lo).max() / P)))
        ksub1 = max(ksub1, int(math.ceil(cnt.max() / P)) if len(sel) else 1)
        per_dev.append((sel, dloc, blk, cnt))

    ksub = (klo_max, khi_max) if ant else ksub1
    kt = klo_max + khi_max if ant else ksub1
    nsub = nblk * kt
    cap = kt * P
    in_edge = []
    for dcore in range(CORES):
        sel, dloc, blk, cnt = per_dev[dcore]
        srcx = np.zeros((nblk, cap), dtype=np.int32)
        dl = np.full((nblk, cap), 255.0, dtype=np.float32)
        eav = np.zeros((nblk, cap), dtype=np.float32)
        starts = np.concatenate([[0], np.cumsum(cnt)])
        for b in range(nblk):
            s0, s1 = starts[b], starts[b + 1]
            seg = sel[s0:s1]
            if ant:
                # lo edges fill subtiles [0, klo_max), hi the rest
                slo = seg[lo_edge[seg]]
                shi = seg[~lo_edge[seg]]
                for part, base in ((slo, 0), (shi, klo_max * P)):
                    so = np.argsort(src_p[part], kind="stable")
                    part = part[so]
                    m = len(part)
                    srcx[b, base:base + m] = src_p[part]
                    dl[b, base:base + m] = (dst_p[part] - dcore * npad) - b * P
                    eav[b, base:base + m] = ea[part]
            else:
                m = s1 - s0
                # sort the block's edges by source row for gather locality
                so = np.argsort(src_p[seg], kind="stable")
                seg = seg[so]
                srcx[b, :m] = src_p[seg]
                dl[b, :m] = (dst_p[seg] - dcore * npad) - b * P
                eav[b, :m] = ea[seg]
        # [nblk, cap] -> [P, nsub]: subtile k of block b at col b*kt+k,
        # edge slot p on partition p
        def to_tiles(a):
            return np.ascontiguousarray(
                a.reshape(nblk, kt, P).transpose(2, 0, 1).reshape(P, nsub))
        # wrapped int16 indices for dma_gather: per block, gather order
        # i = k*128 + p, stored at [i % 16, i // 16], 16-row pattern
        # replicated across all 128 partitions; hi indices offset by -split
        sx = srcx.reshape(nblk, kt * P).astype(np.int64)
        if ant:
            sx = sx - (sx >= split) * split
        gw = sx.reshape(nblk, kt * P // 16, 16).transpose(2, 0, 1)
        gidx = np.ascontiguousarray(
            np.tile(gw, (8, 1, 1)).reshape(P, nsub * 8)).astype(np.int16)
        in_edge.append({
            "src_idx": to_tiles(srcx),
            "gidx": gidx,
            "dst_loc": to_tiles(dl),
            "ea": to_tiles(eav),
        })
    return nloc, npad, ksub, in_edge, pos, order


def make_in_maps(inputs):
    x = np.asarray(inputs["x"], dtype=np.float32)
    nloc, npad, ksub, in_edge, pos, order = preprocess(
        x, inputs["edge_index"], inputs["edge_attr"])

    def f32(name):
        return np.asarray(inputs[name], dtype=np.float32)

    w_gat = f32("gat_W")
    att_srcT = np.ascontiguousarray(f32("att_src").transpose(0, 2, 1))
    att_dstT = np.ascontiguousarray(f32("att_dst").transpose(0, 2, 1))
    edge_WT = np.ascontiguousarray(
        f32("edge_W").reshape(DEPTH, H, C).transpose(0, 2, 1))
    att_edgeT = np.ascontiguousarray(f32("att_edge").transpose(0, 2, 1))
    gat_biasT = np.ascontiguousarray(
        f32("gat_bias").reshape(DEPTH, H, C).transpose(0, 2, 1))
    ff_b1r = np.ascontiguousarray(f32("ff_b1").reshape(DEPTH, 1, MLP))


    shared = {
        "gat_W": w_gat,
        "att_srcT": att_srcT, "att_dstT": att_dstT,
        "edge_WT": edge_WT, "att_edgeT": att_edgeT, "gat_biasT": gat_biasT,
        "qf_W": f32("qf_W"), "qf_b": f32("qf_b"),
        "ln1_gT": f32("ln1_g")[:, :, None], "ln1_bT": f32("ln1_b")[:, :, None],
        "ln2_gT": f32("ln2_g")[:, :, None], "ln2_bT": f32("ln2_b")[:, :, None],
        "ff_W1": f32("ff_W1"), "ff_b1r": ff_b1r,
        "ff_W2": f32("ff_W2"), "ff_b2": f32("ff_b2"),
    }
    in_maps = []
    for dcore in range(CORES):
        xs = x[dcore * nloc:(dcore + 1) * nloc]
        xp = np.zeros((npad, D), np.float32)
        valid = order[dcore] >= 0
        xp[valid] = xs[order[dcore][valid]]
        m = {"x": xp, **in_edge[dcore], **shared}
        in_maps.append(m)
    return nloc, npad, ksub, in_maps, pos


# ----------------------------------------------------------------------------
# PJRT runner (build once, reuse executable)
# ----------------------------------------------------------------------------

_CACHE = {}


def _make_runner(nc, n_cores):
    import hashlib
    import os
    import time
    import jax
    import jax.numpy as jnp
    from jax.sharding import Mesh, PartitionSpec, NamedSharding
    from jax.experimental.shard_map import shard_map
    from concourse.bass2jax import _bass_exec_p, partition_id_tensor

    # The PJRT-level MODULE hash that keys the neuronxcc NEFF cache does not
    # cover the bass program carried in the custom-call backend_config, so two
    # different kernels can collide on the same cached NEFF. Namespace the
    # cache by a digest of the BIR to make it content-sensitive.
    bir_digest = hashlib.sha1(nc.to_json_bytes()).hexdigest()[:20]
    cache_url = f"/root/.neuron-compile-cache-bass/{bir_digest}"

    def _set_cache():
        os.environ["NEURON_COMPILE_CACHE_URL"] = cache_url

    in_names, out_names, out_avals = [], [], []
    pname = nc.partition_id_tensor.name if nc.partition_id_tensor else None
    for alloc in nc.m.functions[0].allocations:
        if not isinstance(alloc, mybir.MemoryLocationSet):
            continue
        nm = alloc.memorylocations[0].name
        if alloc.kind == "ExternalInput" and nm != pname:
            in_names.append(nm)
        elif alloc.kind == "ExternalOutput":
            out_names.append(nm)
            out_avals.append(jax.core.ShapedArray(
                tuple(alloc.tensor_shape), mybir.dt.np(alloc.dtype)))
    n_params, n_outs = len(in_names), len(out_names)
    all_names = in_names + out_names + ([pname] if pname else [])
    donate = tuple(range(n_params, n_params + n_outs))

    def _body(*args):
        operands = list(args)
        if pname:
            operands.append(partition_id_tensor())
        return tuple(_bass_exec_p.bind(
            *operands, out_avals=tuple(out_avals), in_names=tuple(all_names),
            out_names=tuple(out_names), lowering_input_output_aliases=(),
            sim_require_finite=False, sim_require_nnan=False, nc=nc))

    if os.environ.get("BASS_SIM") == "1":
        devices = jax.devices("cpu")
        if len(devices) < n_cores:
            raise RuntimeError(
                f"BASS_SIM needs {n_cores} cpu devices; set "
                f"XLA_FLAGS=--xla_force_host_platform_device_count={n_cores}")
        devices = devices[:n_cores]
    else:
        devices = jax.devices()[:n_cores]
    mesh = Mesh(np.asarray(devices), ("core",))
    sharded = jax.jit(
        shard_map(_body, mesh=mesh,
                  in_specs=(PartitionSpec("core"),) * (n_params + n_outs),
                  out_specs=(PartitionSpec("core"),) * n_outs,
                  check_rep=False),
        donate_argnums=donate, keep_unused=True)
    shard = NamedSharding(mesh, PartitionSpec("core"))
    zero_shapes = [(n_cores * a.shape[0], *a.shape[1:]) for a in out_avals]
    zero_dtypes = [a.dtype for a in out_avals]
    make_zeros = jax.jit(
        lambda: tuple(jnp.zeros(s, d) for s, d in zip(zero_shapes, zero_dtypes)),
        out_shardings=tuple(shard for _ in out_avals))

    def run(in_maps, n_timing_iters=0, return_caller=False):
        concat_in = [
            jax.device_put(np.concatenate(
                [np.ascontiguousarray(m[nm]) for m in in_maps], axis=0), shard)
            for nm in in_names
        ]

        def call():
            _set_cache()
            zeros = make_zeros()
            jax.block_until_ready(zeros)
            t0 = time.perf_counter()
            out = sharded(*concat_in, *zeros)
            jax.block_until_ready(out)
            return out, time.perf_counter() - t0

        out_arrs = None
        for attempt in range(3):
            try:
                out_arrs, _ = call()
                break
            except Exception:
                if attempt == 2:
                    raise
                time.sleep(10.0)
        best = None
        for _ in range(n_timing_iters):
            out_arrs, dt = call()
            best = dt if best is None else min(best, dt)
        results = [
            {nm: np.asarray(out_arrs[i]).reshape(n_cores, *out_avals[i].shape)[c]
             for i, nm in enumerate(out_names)}
            for c in range(n_cores)
        ]
        if return_caller:
            return results, (lambda: call()[1] * 1e9)
        return results, (None if best is None else best * 1e9)

    return run


def run_kernel(inputs, n_timing_iters=0):
    nloc, npad, ksub, in_maps, pos = make_in_maps(inputs)
    key = (npad, ksub)
    if key not in _CACHE:
        nc = build_nc(npad, ksub)
        _CACHE[key] = _make_runner(nc, CORES)
    results, best_ns = _CACHE[key](in_maps, n_timing_iters=n_timing_iters)
    out = np.concatenate(
        [results[c]["x_out"][pos[c]] for c in range(CORES)], axis=0)
    return out, best_ns


def kernel(**inputs):
    out, _ = run_kernel(inputs)
    return out


# revision 60
# speedup vs baseline: 1.3919x; 1.3919x over previous
"""GAT-Transformer (2-layer) distributed Bass kernel for 8 Trainium2 NeuronCores.

Sharding: nodes partitioned across 8 cores (5000/core, padded to 5120 = 40
blocks x 128), LPT-permuted so per-block in-degree loads balance. Edges
partitioned by destination node; segment-softmax and scatter-aggregate stay
device-local.

Key algebraic trick: the GAT aggregation is moved BEFORE the weight matmul.
Since out[dst] = sum_e attn_e * (xn[src_e] @ W) = (sum_e attn_e * xn[src_e]) @ W,
each core only shares/gathers the 128-dim LN-normalized features xn (in fp8,
plus 4 bf16 alpha_src logits: 136B/row) instead of 512-dim bf16 h rows: the
AllGather and per-edge gather shrink ~8x and the stage-A h matmul disappears.
The per-head W_h @ qf_W_h product folds into one [128,128] M_h on device, so
the GAT output never materializes: x += sum_h z_h @ M_h.

Pipeline: stage A (LN + alpha logits) for layer d+1 is emitted inside layer
d's edge loop right after each block's FF, so the chunked AllGather of layer
d+1 overlaps layer d's edge phase. After each AG chunk, a local DRAM->DRAM
DMA expands rows to a 256B stride for dma_gather (the ANT SWDGE gather; one
call per (block, src-half), int16 indices wrapped 16-wide). Per dst block:
one-hot scatter matrices S via iota+is_equal, S^T via TensorE transpose gives
per-edge alpha_dst and per-edge 1/den, attention weights scale S (one DVE op
per subtile over all 4 heads), TensorE accumulates zT = xn_g^T @ S_attn in
PSUM. Softmax max-subtraction is skipped: alpha is bounded (|a| ~ 1), exp()
cannot overflow, result matches reference well below tolerance.

LayerNorm gains/biases fold into adjacent weights on device; ff_b1 rides a
rank-1 matmul so gelu is a single bias-free [P,512] op; LN variance uses
E[x^2]-mu^2 with the free-axis sum on the ACT accumulator port.
Host-side preprocessing only reorders/partitions data (edge sort by dst then
src, LPT node permutation, index remapping, weight transposes).
"""
import math

import numpy as np
import ml_dtypes

import concourse.bacc as bacc
import concourse.mybir as mybir
import concourse.tile as tile
from concourse.bass import IndirectOffsetOnAxis
from concourse.masks import make_identity

# model dims (fixed by the problem)
D = 128         # model dim
H = 4           # heads
C = 128         # per-head channels
HC = H * C      # 512
MLP = 512
DEPTH = 2
NEG_SLOPE = 0.2
EPS_SM = 1e-16
EPS_LN = 1e-5

CORES = 8
P = 128

# scheduling knobs
TUNE = {"sb": 8, "hg": 8, "pgat": 1, "ptr": 2, "pmm": 3, "palp": 1, "pdn": 1}
import os as _os
GATHER_MODE = _os.environ.get("GATHER_MODE", "ant")  # ant | sub | block
FP8 = _os.environ.get("H_FP8", "0") == "1"
NCHUNK = int(_os.environ.get("NCHUNK", "2"))
CHUNKS = _os.environ.get("CHUNKS", "")  # e.g. "16,16,8": blocks per AG chunk

BF16 = mybir.dt.bfloat16
F32 = mybir.dt.float32
F8 = mybir.dt.float8e4
I32 = mybir.dt.int32
I16 = mybir.dt.int16
HDT = F8 if FP8 else BF16
HB = (D + 8) if FP8 else (D + H)   # gathered row cols: xn + alpha_src(bf16)
EXP = 256                          # expanded row elems for dma_gather
AF = mybir.ActivationFunctionType
ALU = mybir.AluOpType
AX = mybir.AxisListType


def _asrc_view(hg_ap):
    """bf16 view of the packed alpha_src columns of a gathered-row slice."""
    if FP8:
        return hg_ap[..., D:D + 8].bitcast(BF16)
    return hg_ap[..., D:D + H]


# ----------------------------------------------------------------------------
# device kernel builder
# ----------------------------------------------------------------------------

def build_nc(npad, ksub, rep=1, do_edges=True, do_gather=True, do_dense=True,
             do_adst=True, do_msg=True, do_coll=True):
    """Build the Bass program for one core. npad: padded local node count
    (multiple of 128); ksub: 128-edge subtiles per 128-node block — an int
    (indirect-DMA per-subtile gather) or a (ksub_lo, ksub_hi) tuple
    (dma_gather with the node range split for int16 indices).
    rep>1 repeats the whole network body (timing only). do_* flags disable
    pipeline stages for perf bisection (results wrong when False)."""
    if isinstance(ksub, tuple):
        ksub_lo, ksub_hi = ksub
        ksub = ksub_lo + ksub_hi
        ant_gather = True
    else:
        ant_gather = False
    nblk = npad // P
    nsub = nblk * ksub
    ng = CORES * npad  # global padded nodes
    split = (CORES // 2) * npad  # lo/hi node-range split (int16 idx limit)
    R = DEPTH * rep
    if not ant_gather:
        cbs = [nblk]
    elif CHUNKS:
        cbs = [int(t) for t in CHUNKS.split(",")]
    elif nblk == 40:
        cbs = [14, 14, 12]
    else:
        nch = min(NCHUNK, nblk)
        cbs = [nblk // nch] * nch
        cbs[-1] += nblk - sum(cbs)
    assert sum(cbs) == nblk
    nck = len(cbs)
    cb0 = [sum(cbs[:i]) for i in range(nck)]  # chunk start block

    from contextlib import ExitStack
    nc = bacc.Bacc(None, target_bir_lowering=False, debug=False)
    with tile.TileContext(nc) as tc, ExitStack() as es:
        dram = es.enter_context(tc.tile_pool(name="dram", bufs=1, space="DRAM"))
        const = es.enter_context(tc.tile_pool(name="const", bufs=1))
        wpool = es.enter_context(tc.tile_pool(name="wpool", bufs=1))
        sb = es.enter_context(tc.tile_pool(name="sb", bufs=TUNE["sb"]))
        hg = es.enter_context(tc.tile_pool(name="hg", bufs=TUNE["hg"]))
        pgat = es.enter_context(tc.tile_pool(name="pgat", bufs=TUNE["pgat"], space="PSUM"))
        palpha = es.enter_context(tc.tile_pool(name="palpha", bufs=TUNE["palp"], space="PSUM"))
        pdn = es.enter_context(tc.tile_pool(name="pdn", bufs=TUNE["pdn"], space="PSUM"))
        ptr = es.enter_context(tc.tile_pool(name="ptr", bufs=TUNE["ptr"], space="PSUM"))
        pmm = es.enter_context(tc.tile_pool(name="pmm", bufs=TUNE["pmm"], space="PSUM"))
        p512 = pmm
        p128 = pmm

        # ---- I/O ------------------------------------------------------------
        def einp(name, shape, dtype=F32):
            return dram.tile(shape, dtype, kind="ExternalInput", name=name,
                             uniquify=False)

        x_in = einp("x", [npad, D])
        src_idx = einp("src_idx", [P, nsub], I32)      # global padded row
        gidx_in = einp("gidx", [P, nsub * 8], I16)     # wrapped idx, dma_gather
        dst_loc = einp("dst_loc", [P, nsub])           # dst within block, 255=pad
        ea_in = einp("ea", [P, nsub])                  # edge_attr, sorted order
        w_gat = einp("gat_W", [DEPTH, D, HC])
        a_srcT = einp("att_srcT", [DEPTH, C, H])
        a_dstT = einp("att_dstT", [DEPTH, C, H])
        ew_T = einp("edge_WT", [DEPTH, C, H])
        ae_T = einp("att_edgeT", [DEPTH, C, H])
        gb_T = einp("gat_biasT", [DEPTH, C, H])
        w_qf = einp("qf_W", [DEPTH, HC, D])
        b_qf = einp("qf_b", [DEPTH, D])
        ln1gT = einp("ln1_gT", [DEPTH, D, 1])
        ln1bT = einp("ln1_bT", [DEPTH, D, 1])
        ln2gT = einp("ln2_gT", [DEPTH, D, 1])
        ln2bT = einp("ln2_bT", [DEPTH, D, 1])
        w_f1 = einp("ff_W1", [DEPTH, D, MLP])
        b_f1r = einp("ff_b1r", [DEPTH, 1, MLP])
        w_f2 = einp("ff_W2", [DEPTH, MLP, D])
        b_f2 = einp("ff_b2", [DEPTH, D])

        x_out = dram.tile([npad, D], F32, kind="ExternalOutput", name="x_out",
                          uniquify=False)

        h_locs = [dram.tile([npad, HB], HDT, name=f"h_loc{d}")
                  for d in range(R)]
        if ant_gather:
            h_fullc = [[dram.tile([CORES * cbs[c] * P, HB], HDT,
                                  addr_space="Shared" if do_coll else "Local",
                                  name=f"h_full{d}_{c}")
                        for c in range(nck)] for d in range(R)]
            h_exps = [dram.tile([ng, EXP], HDT, name=f"h_exp{d}")
                      for d in range(R)]
        else:
            h_fulls = [dram.tile([ng, HB], HDT,
                                 addr_space="Shared" if do_coll else "Local",
                                 name=f"h_full{d}")
                       for d in range(R)]

        # ---- static constants ----------------------------------------------
        iota_i = const.tile([P, P], I32)
        nc.gpsimd.iota(iota_i[:], pattern=[[1, P]], base=0, channel_multiplier=0)
        iota_bf = const.tile([P, P], BF16)
        nc.vector.tensor_copy(out=iota_bf[:], in_=iota_i[:])
        ident_bf = const.tile([P, P], BF16)
        make_identity(nc, ident_bf)
        ones1_bf = const.tile([1, P], BF16)
        nc.vector.memset(ones1_bf[:], 1.0)
        one11_bf = const.tile([1, 1], BF16)
        nc.vector.memset(one11_bf[:], 1.0)
        epsln = const.tile([P, 1], F32)
        nc.vector.memset(epsln[:], EPS_LN)

        x_sb = const.tile([P, nblk, D], F32)
        nc.sync.dma_start(out=x_sb[:], in_=x_in[:].rearrange("(b p) f -> p b f", p=P))
        if ant_gather:
            gidx_sb = const.tile([P, nsub * 8], I16)
            nc.sync.dma_start(out=gidx_sb[:], in_=gidx_in[:])
        else:
            srcx_sb = const.tile([P, nsub], I32)
            nc.sync.dma_start(out=srcx_sb[:], in_=src_idx[:])
        dstl_bf = const.tile([P, nsub], BF16)
        nc.gpsimd.dma_start(out=dstl_bf[:], in_=dst_loc[:])
        dstl_f = const.tile([P, nsub], F32)
        nc.sync.dma_start(out=dstl_f[:], in_=dst_loc[:])
        ea_sb = const.tile([P, nsub], F32)
        nc.sync.dma_start(out=ea_sb[:], in_=ea_in[:])
        adst_par = [const.tile([P, nblk, H], BF16, name=f"adst{par}")
                    for par in range(DEPTH)]

        # ---------------- per-layer weight prep (both layers upfront) -------
        ws = []
        for d in range(DEPTH):
            W = {}

            def wt(shape, dtype, nm):
                return wpool.tile(shape, dtype, name=f"{nm}{d}")

            w_bf = wt([D, HC], BF16, "w_bf")
            nc.gpsimd.dma_start(out=w_bf[:], in_=w_gat[d])   # cast f32->bf16
            qfw_bf = wt([P, H, D], BF16, "qfw_bf")
            nc.gpsimd.dma_start(out=qfw_bf[:],
                                in_=w_qf[d].rearrange("(c p) n -> p c n", p=P))
            f1w_bf = wt([D, MLP], BF16, "f1w_bf")
            nc.gpsimd.dma_start(out=f1w_bf[:], in_=w_f1[d])
            f2w_bf = wt([P, H, D], BF16, "f2w_bf")
            nc.gpsimd.dma_start(out=f2w_bf[:],
                                in_=w_f2[d].rearrange("(c p) n -> p c n", p=P))
            att_bf = wt([C, 4, H], BF16, "att_bf")  # src,dst,ew,ae cols
            nc.gpsimd.dma_start(out=att_bf[:, 0, :], in_=a_srcT[d])
            nc.gpsimd.dma_start(out=att_bf[:, 1, :], in_=a_dstT[d])
            nc.gpsimd.dma_start(out=att_bf[:, 2, :], in_=ew_T[d])
            nc.gpsimd.dma_start(out=att_bf[:, 3, :], in_=ae_T[d])
            gbT_bf = wt([C, H], BF16, "gbT_bf")
            nc.gpsimd.dma_start(out=gbT_bf[:], in_=gb_T[d])
            qfb_bf = wt([1, D], BF16, "qfb_bf")
            nc.gpsimd.dma_start(out=qfb_bf[:], in_=b_qf[d:d + 1, :])
            f2b_bf = wt([1, D], BF16, "f2b_bf")
            nc.gpsimd.dma_start(out=f2b_bf[:], in_=b_f2[d:d + 1, :])
            f1b_bf = wt([1, MLP], BF16, "f1b_bf")
            nc.gpsimd.dma_start(out=f1b_bf[:], in_=b_f1r[d])
            g1c = wt([D, 1], F32, "g1c")
            nc.sync.dma_start(out=g1c[:], in_=ln1gT[d])
            b1c = wt([D, 1], BF16, "b1c")
            nc.gpsimd.dma_start(out=b1c[:], in_=ln1bT[d])
            g2c = wt([D, 1], F32, "g2c")
            nc.sync.dma_start(out=g2c[:], in_=ln2gT[d])
            b2c = wt([D, 1], BF16, "b2c")
            nc.gpsimd.dma_start(out=b2c[:], in_=ln2bT[d])

            # fold LN1 gain into W (h = xn @ w_s + b1^T W)
            w_s = wt([D, HC], BF16, "w_s")
            nc.vector.tensor_scalar_mul(out=w_s[:], in0=w_bf[:], scalar1=g1c[:, :1])

            # W transposed per head (unscaled, for Wa) and scaled (for M)
            wT_bf = wt([C, H, D], BF16, "wT_bf")
            wsT_bf = wt([C, H, D], BF16, "wsT_bf")
            for h in range(H):
                pst = ptr.tile([P, P], BF16, space="PSUM", name="pst")
                nc.tensor.transpose(out=pst[:], in_=w_bf[:, h * C:(h + 1) * C],
                                    identity=ident_bf[:])
                nc.any.tensor_copy(out=wT_bf[:, h, :], in_=pst[:])
                pst2 = ptr.tile([P, P], BF16, space="PSUM", name="pst")
                nc.tensor.transpose(out=pst2[:], in_=w_s[:, h * C:(h + 1) * C],
                                    identity=ident_bf[:])
                nc.any.tensor_copy(out=wsT_bf[:, h, :], in_=pst2[:])
            # Wa[:, 0:4] = W_h @ a_src_h ; [:, 4:8] = W_h @ a_dst_h
            pwa = p128.tile([D, 2 * H], F32, space="PSUM", name="p128t", tag="pmmt")
            for h in range(H):
                nc.tensor.matmul(out=pwa[:, h:h + 1], lhsT=wT_bf[:, h, :],
                                 rhs=att_bf[:, 0, h:h + 1], start=True, stop=True)
                nc.tensor.matmul(out=pwa[:, H + h:H + h + 1], lhsT=wT_bf[:, h, :],
                                 rhs=att_bf[:, 1, h:h + 1], start=True, stop=True)
            wa_bf = wt([D, 2 * H], BF16, "wa_bf")
            nc.any.tensor_copy(out=wa_bf[:], in_=pwa[:])
            # fold LN1 gain into Wa; LN1 bias becomes rank-1 rows
            wa_s = wt([D, 2 * H], BF16, "wa_s")
            nc.vector.tensor_scalar_mul(out=wa_s[:], in0=wa_bf[:], scalar1=g1c[:, :1])
            pbwa = p128.tile([1, 2 * H], F32, space="PSUM", name="pbwa", tag="pmmt")
            nc.tensor.matmul(out=pbwa[:], lhsT=b1c[:], rhs=wa_bf[:],
                             start=True, stop=True)
            bwa_row = wt([1, 2 * H], BF16, "bwa_row")
            nc.any.tensor_copy(out=bwa_row[:], in_=pbwa[:])
            # per-head b1^T @ W_h as columns (for the qf-folded bias)
            pbwc = p128.tile([C, H], F32, space="PSUM", name="pbwc", tag="pmmt")
            for h in range(H):
                nc.tensor.matmul(out=pbwc[:, h:h + 1],
                                 lhsT=w_bf[:, h * C:(h + 1) * C], rhs=b1c[:],
                                 start=True, stop=True)
            bwc_bf = wt([C, H], BF16, "bwc_bf")
            nc.any.tensor_copy(out=bwc_bf[:], in_=pbwc[:])
            # fold LN2 gain into ff_W1; LN2 bias + ff_b1 -> rank-1 row
            f1w_s = wt([D, MLP], BF16, "f1w_s")
            nc.vector.tensor_scalar_mul(out=f1w_s[:], in0=f1w_bf[:], scalar1=g2c[:, :1])
            pbw1 = p512.tile([1, MLP], F32, space="PSUM", name="pbw1", tag="pmmt")
            nc.tensor.matmul(out=pbw1[:], lhsT=b2c[:], rhs=f1w_bf[:],
                             start=True, stop=True)
            bw1_row = wt([1, MLP], BF16, "bw1_row")
            nc.vector.tensor_tensor(out=bw1_row[:], in0=pbw1[:], in1=f1b_bf[:],
                                    op=ALU.add)

            # M_h = (diag(g1) W)_h @ qf_W_h   [128, 128] per head
            m_bf = wt([P, H, D], BF16, "m_bf")
            for h in range(H):
                pM = p128.tile([P, D], F32, space="PSUM", name="p128m", tag="pmmt")
                nc.tensor.matmul(out=pM[:], lhsT=wsT_bf[:, h, :],
                                 rhs=qfw_bf[:, h, :], start=True, stop=True)
                nc.any.tensor_copy(out=m_bf[:, h, :], in_=pM[:])

            # we_dot[h] = <edge_W_h, a_edge_h>, replicated [P, ksub*H]
            pwe = p128.tile([1, H], F32, space="PSUM", name="p128t2", tag="pmmt")
            for h in range(H):
                nc.tensor.matmul(out=pwe[:, h:h + 1], lhsT=att_bf[:, 2, h:h + 1],
                                 rhs=att_bf[:, 3, h:h + 1], start=True, stop=True)
            wd_row = wt([1, H], BF16, "wd_row")
            nc.any.tensor_copy(out=wd_row[:], in_=pwe[:])
            wd_rep = wt([1, ksub, H], BF16, "wd_rep")
            nc.vector.tensor_copy(
                out=wd_rep[:],
                in_=wd_row[:].unsqueeze(1).to_broadcast([1, ksub, H]))
            pwr = p512.tile([P, ksub * H], F32, space="PSUM", name="p512t", tag="pmmt")
            nc.tensor.matmul(out=pwr[:], lhsT=ones1_bf[:],
                             rhs=wd_rep[:].rearrange("a k h -> a (k h)"),
                             start=True, stop=True)
            wedot = wt([P, ksub * H], F32, "wedot")
            nc.any.tensor_copy(out=wedot[:], in_=pwr[:])

            # qfb_eff = gat_bias @ qf_W + qf_b   -> [1, D]
            pqb = p128.tile([1, D], F32, space="PSUM", name="p128t3", tag="pmmt")
            for cch in range(H):
                nc.tensor.matmul(out=pqb[:], lhsT=gbT_bf[:, cch:cch + 1],
                                 rhs=qfw_bf[:, cch, :], start=(cch == 0), stop=False)
            nc.tensor.matmul(out=pqb[:], lhsT=one11_bf[:], rhs=qfb_bf[:],
                             start=False, stop=True)
            qfbe = wt([1, D], BF16, "qfbe")
            nc.any.tensor_copy(out=qfbe[:], in_=pqb[:])
            # (b1^T W) @ qf_W broadcast to all partitions; applied per dst
            # gated by [deg>0] since sum_e attn = 0 for isolated nodes
            pqw = p128.tile([1, D], F32, space="PSUM", name="p128t4", tag="pmmt")
            for cch in range(H):
                nc.tensor.matmul(out=pqw[:], lhsT=bwc_bf[:, cch:cch + 1],
                                 rhs=qfw_bf[:, cch, :], start=(cch == 0),
                                 stop=(cch == H - 1))
            qbw_row = wt([1, D], BF16, "qbw_row")
            nc.any.tensor_copy(out=qbw_row[:], in_=pqw[:])
            pqwb = p128.tile([P, D], F32, space="PSUM", name="p128t5", tag="pmmt")
            nc.tensor.matmul(out=pqwb[:], lhsT=ones1_bf[:], rhs=qbw_row[:],
                             start=True, stop=True)
            qbw_bc = wt([P, D], BF16, "qbw_bc")
            nc.any.tensor_copy(out=qbw_bc[:], in_=pqwb[:])

            W.update(w_s=w_s, wa_s=wa_s, bwa_row=bwa_row, f1w_s=f1w_s,
                     bw1_row=bw1_row, f2w_bf=f2w_bf, f2b_bf=f2b_bf, m_bf=m_bf,
                     wedot=wedot, qfbe=qfbe, qbw_bc=qbw_bc)
            ws.append(W)

        # ---------------- batched layer-norm statistics ----------------------
        # One Sqrt per stage (vs per block): activation-table reloads are
        # ~1.3us each and sqrt/exp/gelu live in different table sets, so
        # per-block LNs force constant reloads. Square/Copy are in EVERY set
        # and never force a reload.
        def ln_stats(name, b0=0, nb=None):
            """LN stats for blocks [b0, b0+nb); returns [P, nb] mu/rstd
            indexed by b - b0."""
            nb = nblk if nb is None else nb
            xv = x_sb[:, b0:b0 + nb, :]
            sx = sb.tile([P, nb], F32, name=f"sx{name}", bufs=2)
            nc.vector.tensor_reduce(out=sx[:].unsqueeze(2),
                                    in_=xv, axis=AX.X, op=ALU.add)
            sx2 = sb.tile([P, nb], F32, name=f"sx2{name}", bufs=2)
            for b in range(nb):
                sq = sb.tile([P, D], F32, name=f"sq{name}")
                nc.scalar.activation(out=sq[:], in_=x_sb[:, b0 + b, :],
                                     func=AF.Square,
                                     accum_out=sx2[:, b:b + 1])
            mu = sb.tile([P, nb], F32, name=f"mu{name}", bufs=2)
            nc.vector.tensor_scalar_mul(out=mu[:], in0=sx[:], scalar1=1.0 / D)
            mu2 = sb.tile([P, nb], F32, name=f"mu2{name}")
            nc.vector.tensor_tensor(out=mu2[:], in0=mu[:], in1=mu[:], op=ALU.mult)
            var = sb.tile([P, nb], F32, name=f"var{name}")
            nc.vector.tensor_scalar(out=var[:], in0=sx2[:], scalar1=1.0 / D,
                                    scalar2=None, op0=ALU.mult)
            nc.vector.tensor_tensor(out=var[:], in0=var[:], in1=mu2[:],
                                    op=ALU.subtract)
            std = sb.tile([P, nb], F32, name=f"std{name}")
            nc.scalar.activation(out=std[:], in_=var[:], func=AF.Sqrt,
                                 bias=epsln[:, :1])
            rstd = sb.tile([P, nb], F32, name=f"rstd{name}", bufs=2)
            nc.vector.reciprocal(out=rstd[:], in_=std[:])
            return mu, rstd, b0

        def ln_apply(b, stats, name, xn_out=None):
            """xn = (x - mu)*rstd for one block + transposed copy."""
            mu, rstd, b0 = stats
            if xn_out is None:
                xn_out = sb.tile([P, D], BF16, name=f"xn{name}")
            nc.vector.tensor_scalar(out=xn_out[:], in0=x_sb[:, b, :],
                                    scalar1=mu[:, b - b0:b - b0 + 1],
                                    scalar2=rstd[:, b - b0:b - b0 + 1],
                                    op0=ALU.subtract, op1=ALU.mult)
            pst = ptr.tile([P, P], BF16, space="PSUM", name="pst")
            nc.tensor.transpose(out=pst[:], in_=xn_out[:], identity=ident_bf[:])
            xnT = sb.tile([P, P], BF16, name=f"xnT{name}")
            nc.any.tensor_copy(out=xnT[:], in_=pst[:])
            return xnT, xn_out

        # ---------------- stage A: xn + alpha logits for one block ----------
        def stage_a(rd, b, stats):
            Wd = ws[rd % DEPTH]
            h_sb = sb.tile([P, HB], HDT, name="h_sb")
            if FP8:
                xnT, xn_bf = ln_apply(b, stats, "A")
                nc.vector.tensor_copy(out=h_sb[:, 0:D], in_=xn_bf[:])
            else:
                xnT, _ = ln_apply(b, stats, "A", xn_out=h_sb[:, 0:D])
            pa8 = p128.tile([P, 2 * H], F32, space="PSUM", name="pa8", tag="pmmt")
            nc.tensor.matmul(out=pa8[:], lhsT=xnT[:], rhs=Wd["wa_s"][:],
                             start=True, stop=False)
            nc.tensor.matmul(out=pa8[:], lhsT=ones1_bf[:], rhs=Wd["bwa_row"][:],
                             start=False, stop=True)
            nc.any.tensor_copy(out=_asrc_view(h_sb)[:, :], in_=pa8[:, 0:H])
            nc.any.tensor_copy(out=adst_par[rd % DEPTH][:, b, :],
                               in_=pa8[:, H:2 * H])
            nc.sync.dma_start(out=h_locs[rd][b * P:(b + 1) * P, :], in_=h_sb[:])

        # ---------------- AG chunk + row expand ------------------------------
        def emit_ag(rd, ck):
            h_loc = h_locs[rd]
            if ant_gather:
                hf = h_fullc[rd][ck]
                r0, r1 = cb0[ck] * P, (cb0[ck] + cbs[ck]) * P
                if do_coll:
                    nc.gpsimd.collective_compute(
                        "AllGather", ALU.bypass,
                        replica_groups=[list(range(CORES))],
                        ins=[h_loc[r0:r1, :].opt()],
                        outs=[hf[:].opt()],
                    )
                else:
                    nc.sync.dma_start(out=hf[0:r1 - r0, :], in_=h_loc[r0:r1, :])
                exp_view = h_exps[rd][:].rearrange("(r n) f -> r n f", r=CORES)
                nc.sync.dma_start(
                    out=exp_view[:, r0:r1, 0:HB],
                    in_=hf[:].rearrange("(r n) f -> r n f", r=CORES))
            else:
                if do_coll:
                    nc.gpsimd.collective_compute(
                        "AllGather", ALU.bypass,
                        replica_groups=[list(range(CORES))],
                        ins=[h_loc[:].opt()],
                        outs=[h_fulls[rd][:].opt()],
                    )
                else:
                    nc.sync.dma_start(out=h_fulls[rd][0:npad, :], in_=h_loc[:])

        # ---------------- tail: ff for one block -----------------------------
        def emit_ff(rd, b, stats):
            if not do_dense:
                return
            Wd = ws[rd % DEPTH]
            xn2T, _ = ln_apply(b, stats, "B")
            pa1 = p512.tile([P, MLP], F32, space="PSUM", name="pa1", tag="pmmt")
            a1T = sb.tile([P, H, P], BF16, name="a1T")
            for cch in range(MLP // P):
                nc.tensor.matmul(out=pa1[:, cch * P:(cch + 1) * P],
                                 lhsT=Wd["f1w_s"][:, cch * P:(cch + 1) * P],
                                 rhs=xn2T[:], start=True, stop=False)
                nc.tensor.matmul(out=pa1[:, cch * P:(cch + 1) * P],
                                 lhsT=Wd["bw1_row"][:, cch * P:(cch + 1) * P],
                                 rhs=ones1_bf[:], start=False, stop=True)
            nc.scalar.activation(
                out=a1T[:].rearrange("p h q -> p (h q)"), in_=pa1[:],
                func=AF.Gelu_apprx_tanh)
            pf2 = p128.tile([P, D], F32, space="PSUM", name="pf2", tag="pmmt")
            for cch in range(MLP // P):
                nc.tensor.matmul(out=pf2[:], lhsT=a1T[:, cch, :],
                                 rhs=Wd["f2w_bf"][:, cch, :],
                                 start=(cch == 0), stop=False)
            nc.tensor.matmul(out=pf2[:], lhsT=ones1_bf[:], rhs=Wd["f2b_bf"][:],
                             start=False, stop=True)
            nc.vector.tensor_add(out=x_sb[:, b, :], in0=x_sb[:, b, :], in1=pf2[:])

        # ---------------- edge phase + qf for one block ----------------------
        def edge_block(rd, b):
            Wd = ws[rd % DEPTH]
            adst_all = adst_par[rd % DEPTH]
            hgat = hg.tile([P, ksub, EXP if ant_gather else HB], HDT,
                           name="hgat")
            if not do_gather:
                nc.sync.dma_start(
                    out=hgat[:, :, 0:HB],
                    in_=(h_exps[rd] if ant_gather else h_fulls[rd])
                        [0:ksub * P, 0:HB].rearrange("(k p) f -> p k f", p=P))
            elif ant_gather:
                h_exp = h_exps[rd]
                gcols = ksub * P // 16
                g0 = b * gcols
                for (kq0, kq1, base) in ((0, ksub_lo, 0),
                                         (ksub_lo, ksub, split)):
                    kk = kq1 - kq0
                    if kk == 0:
                        continue
                    nc.gpsimd.dma_gather(
                        out_ap=hgat[:, kq0:kq1, :],
                        in_ap=h_exp[base:base + split, :],
                        idxs_ap=gidx_sb[:, g0 + kq0 * 8:g0 + kq1 * 8],
                        num_idxs=kk * P, num_idxs_reg=kk * P,
                        elem_size=EXP)
            elif GATHER_MODE == "block":
                nc.gpsimd.indirect_dma_start(
                    out=hgat[:], out_offset=None, in_=h_fulls[rd][:],
                    in_offset=IndirectOffsetOnAxis(
                        ap=srcx_sb[:, b * ksub:(b + 1) * ksub], axis=0))
            else:
                for k in range(ksub):
                    s = b * ksub + k
                    nc.gpsimd.indirect_dma_start(
                        out=hgat[:, k, :], out_offset=None, in_=h_fulls[rd][:],
                        in_offset=IndirectOffsetOnAxis(
                            ap=srcx_sb[:, s:s + 1], axis=0))
            s_all = sb.tile([P, ksub, P], BF16, name="s_all", bufs=2)
            nc.vector.tensor_tensor(
                out=s_all[:],
                in0=iota_bf[:].unsqueeze(1).to_broadcast([P, ksub, P]),
                in1=dstl_bf[:, b * ksub:(b + 1) * ksub].unsqueeze(2)
                    .to_broadcast([P, ksub, P]),
                op=ALU.is_equal)
            sT_all = sb.tile([P, ksub, P], BF16, name="sT_all", bufs=2)
            palp = palpha.tile([P, ksub * H], F32, space="PSUM", name="palp",
                               tag="palp")
            for k0 in range(0, ksub, 2):
                kp = min(2, ksub - k0)
                pst = ptr.tile([P, 2, P], BF16, space="PSUM", name="pst")
                for j in range(kp):
                    nc.tensor.transpose(out=pst[:, j, :],
                                        in_=s_all[:, k0 + j, :],
                                        identity=ident_bf[:])
                if (k0 // 2) % 2 == 0:
                    nc.vector.tensor_copy(out=sT_all[:, k0:k0 + kp, :],
                                          in_=pst[:, 0:kp, :])
                else:
                    nc.scalar.activation(out=sT_all[:, k0:k0 + kp, :],
                                         in_=pst[:, 0:kp, :], func=AF.Copy)
                if do_adst:
                    for j in range(kp):
                        k = k0 + j
                        nc.tensor.matmul(out=palp[:, k * H:(k + 1) * H],
                                         lhsT=sT_all[:, k, :],
                                         rhs=adst_all[:, b, :],
                                         start=True, stop=True)
            # alpha = asrc + adst + ea*wedot; lrelu; exp (whole block)
            ex_bf = sb.tile([P, ksub * H], BF16, name="ex_bf")
            al1 = sb.tile([P, ksub * H], F32, name="al1")
            if do_adst:
                nc.vector.tensor_tensor(
                    out=al1[:].rearrange("p (k h) -> p k h", h=H),
                    in0=palp[:].rearrange("p (k h) -> p k h", h=H),
                    in1=_asrc_view(hgat)[:, :, :], op=ALU.add)
            else:
                nc.vector.tensor_copy(
                    out=al1[:].rearrange("p (k h) -> p k h", h=H),
                    in_=_asrc_view(hgat)[:, :, :])
            aef = sb.tile([P, ksub, H], F32, name="aef")
            nc.vector.tensor_tensor(
                out=aef[:],
                in0=Wd["wedot"][:].rearrange("p (k h) -> p k h", h=H),
                in1=ea_sb[:, b * ksub:(b + 1) * ksub].unsqueeze(2)
                    .to_broadcast([P, ksub, H]),
                op=ALU.mult)
            al2 = sb.tile([P, ksub * H], F32, name="al2")
            nc.vector.tensor_tensor(
                out=al2[:], in0=al1[:],
                in1=aef[:].rearrange("p k h -> p (k h)"), op=ALU.add)
            lr = sb.tile([P, ksub * H], F32, name="lr")
            nc.vector.scalar_tensor_tensor(
                out=lr[:], in0=al2[:], scalar=NEG_SLOPE, in1=al2[:],
                op0=ALU.mult, op1=ALU.max)
            nc.scalar.activation(out=ex_bf[:], in_=lr[:], func=AF.Exp)

            # denominators: den = S^T @ ex  (per head)
            pd = pdn.tile([P, H], F32, space="PSUM", name="pd")
            for k in range(ksub):
                nc.tensor.matmul(out=pd[:], lhsT=s_all[:, k, :],
                                 rhs=ex_bf[:, k * H:(k + 1) * H],
                                 start=(k == 0), stop=(k == ksub - 1))
            den = sb.tile([P, H], F32, name="den")
            nc.vector.tensor_scalar_add(out=den[:], in0=pd[:], scalar1=EPS_SM)
            ind = sb.tile([P, 1], F32, name="ind")
            nc.vector.tensor_scalar(out=ind[:], in0=pd[:, 0:1],
                                    scalar1=1e30, scalar2=1.0,
                                    op0=ALU.mult, op1=ALU.min)
            rec = sb.tile([P, H], F32, name="rec")
            nc.vector.reciprocal(out=rec[:], in_=den[:])
            rec_bf = sb.tile([P, H], BF16, name="rec_bf")
            nc.vector.tensor_copy(out=rec_bf[:], in_=rec[:])
            # per-edge 1/den via S^T gather-matmul, then attn = ex * rec_e
            prec = palpha.tile([P, ksub * H], F32, space="PSUM", name="prec",
                               tag="palp")
            for k in range(ksub):
                nc.tensor.matmul(out=prec[:, k * H:(k + 1) * H],
                                 lhsT=sT_all[:, k, :], rhs=rec_bf[:],
                                 start=True, stop=True)
            attn = sb.tile([P, ksub * H], F32, name="attn")
            nc.vector.tensor_tensor(out=attn[:], in0=ex_bf[:], in1=prec[:],
                                    op=ALU.mult)

            # zT_h = xn_g^T @ (S * attn_h), accumulated over subtiles.
            # S*attn built directly: (iota == dstl) * attn, one fused
            # tensor_scalar per (subtile, head) -> DVE 4x mode eligible.
            pz = pgat.tile([P, H * P], F32, space="PSUM", name="pz")
            for k in range(ksub):
                sat = sb.tile([P, H, P], BF16, name="sat", bufs=4)
                for h in range(H):
                    nc.vector.tensor_scalar(
                        out=sat[:, h, :], in0=iota_bf[:],
                        scalar1=dstl_f[:, b * ksub + k:b * ksub + k + 1],
                        scalar2=attn[:, k * H + h:k * H + h + 1],
                        op0=ALU.is_equal, op1=ALU.mult)
                rhs = (sat[:].rearrange("p h q -> p (h q)") if do_msg else
                       s_all[:, k, :].unsqueeze(1).to_broadcast([P, H, P])
                       .rearrange("p h q -> p (h q)"))
                nc.tensor.matmul(out=pz[:], lhsT=hgat[:, k, 0:D], rhs=rhs,
                                 start=(k == 0), stop=(k == ksub - 1))
            zt = sb.tile([P, H * P], BF16, name="zt")
            nc.any.tensor_copy(out=zt[:], in_=pz[:])

            if do_dense:
                # qf: pxs[b] = sum_h z_h @ M_h + qfb_eff + [deg>0]*(b1 W qf_W)
                # staged to SBUF; added to x in one batched op per layer
                px = p128.tile([P, D], F32, space="PSUM", name="px", tag="pmmt")
                for h in range(H):
                    nc.tensor.matmul(out=px[:], lhsT=zt[:, h * P:(h + 1) * P],
                                     rhs=Wd["m_bf"][:, h, :],
                                     start=(h == 0), stop=False)
                nc.tensor.matmul(out=px[:], lhsT=ones1_bf[:], rhs=Wd["qfbe"][:],
                                 start=False, stop=True)
                gbw = sb.tile([P, D], BF16, name="gbw")
                nc.any.tensor_scalar_mul(out=gbw[:], in0=Wd["qbw_bc"][:],
                                         scalar1=ind[:, :1])
                nc.vector.tensor_tensor(out=pxs_all[:, b, :], in0=px[:],
                                        in1=gbw[:], op=ALU.add)

        # ---------------- main pipeline --------------------------------------
        # Per layer: E-pass (edges; ACT runs exp only), one batched x += pxs,
        # F-pass (FF; ACT runs gelu only), A-pass (next layer's stage A).
        # Each LN stage shares one batched Sqrt.
        pxs_all = const.tile([P, nblk, D], BF16, name="pxs_all")
        st0 = ln_stats("A")
        for c in range(nck):
            for b in range(cb0[c], cb0[c] + cbs[c]):
                stage_a(0, b, st0)
            emit_ag(0, c)
        # Per chunk of blocks: edge phase -> qf add -> FF -> next stage A ->
        # AG chunk. The AG chunks of layer rd+1 then overlap the remaining
        # edge-phase chunks of layer rd.
        for rd in range(R):
            for c in range(nck):
                b0, bpc = cb0[c], cbs[c]
                if do_edges:
                    for b in range(b0, b0 + bpc):
                        edge_block(rd, b)
                    if do_dense:
                        nc.vector.tensor_tensor(
                            out=x_sb[:, b0:b0 + bpc, :],
                            in0=x_sb[:, b0:b0 + bpc, :],
                            in1=pxs_all[:, b0:b0 + bpc, :], op=ALU.add)
                if do_dense:
                    stf = ln_stats("B", b0, bpc)
                    for b in range(b0, b0 + bpc):
                        emit_ff(rd, b, stf)
                if rd + 1 < R:
                    sta = ln_stats("A", b0, bpc)
                    for b in range(b0, b0 + bpc):
                        stage_a(rd + 1, b, sta)
                    emit_ag(rd + 1, c)

        nc.sync.dma_start(out=x_out[:].rearrange("(b p) f -> p b f", p=P),
                          in_=x_sb[:])
    nc.finalize()
    return nc


# ----------------------------------------------------------------------------
# host-side sharding / preprocessing
# ----------------------------------------------------------------------------

def preprocess(x, edge_index, edge_attr):
    n = x.shape[0]
    e = edge_index.shape[1]
    assert n % CORES == 0
    nloc = n // CORES
    npad = ((nloc + P - 1) // P) * P
    nblk = npad // P

    src = np.asarray(edge_index[0], dtype=np.int64)
    dst = np.asarray(edge_index[1], dtype=np.int64)
    dev = dst // nloc

    split = (CORES // 2) * npad
    ant = GATHER_MODE == "ant"
    lo_edge = src < (CORES // 2) * nloc  # src on cores 0..3 -> row < split

    # LPT-pack local nodes into blocks so per-block in-degree sums balance.
    # For the ant (dma_gather) mode, balance the lo/hi src-range loads
    # jointly since each is padded to its own subtile count.
    # pos[c, i] = padded row of local node i of core c; order[c, r] = local
    # node at padded row r (-1 = hole).
    pos = np.empty((CORES, nloc), dtype=np.int64)
    order = np.full((CORES, npad), -1, dtype=np.int64)
    for c in range(CORES):
        sel_c = dev == c
        dst_c = dst[sel_c] - c * nloc
        deg_lo = np.bincount(dst_c[lo_edge[sel_c]], minlength=nloc)
        deg_hi = np.bincount(dst_c[~lo_edge[sel_c]], minlength=nloc)
        degc = deg_lo + deg_hi
        byd = np.argsort(-degc, kind="stable")
        load_lo = np.zeros(nblk, dtype=np.int64)
        load_hi = np.zeros(nblk, dtype=np.int64)
        fill = np.zeros(nblk, dtype=np.int64)
        for i in byd:
            cand = np.nonzero(fill < P)[0]
            if ant:
                score = np.maximum(load_lo[cand] + deg_lo[i],
                                   load_hi[cand] + deg_hi[i])
            else:
                score = load_lo[cand] + deg_lo[i] + load_hi[cand] + deg_hi[i]
            bsel = cand[np.argmin(score)]
            pos[c, i] = bsel * P + fill[bsel]
            order[c, bsel * P + fill[bsel]] = i
            load_lo[bsel] += deg_lo[i]
            load_hi[bsel] += deg_hi[i]
            fill[bsel] += 1

    # remap to padded (permuted) ids
    src_p = (src // nloc) * npad + pos[src // nloc, src % nloc]
    dst_p = (dst // nloc) * npad + pos[dev, dst % nloc]

    ea = np.asarray(edge_attr, dtype=np.float32).reshape(-1)

    per_dev = []
    klo_max, khi_max, ksub1 = 1, 0, 1
    for dcore in range(CORES):
        sel = np.nonzero(dev == dcore)[0]
        eorder = np.argsort(dst_p[sel], kind="stable")
        sel = sel[eorder]
        dloc = dst_p[sel] - dcore * npad          # [0, npad)
        blk = dloc // P
        cnt = np.bincount(blk, minlength=nblk)
        cnt_lo = np.bincount(blk[lo_edge[sel]], minlength=nblk)
        klo_max = max(klo_max, int(math.ceil(cnt_lo.max() / P)))
        khi_max = max(khi_max,
                      int(math.ceil((cnt - cnt_lo).max() / P)))
        ksub1 = max(ksub1, int(math.ceil(cnt.max() / P)) if len(sel) else 1)
        per_dev.append((sel, dloc, blk, cnt))

    ksub = (klo_max, khi_max) if ant else ksub1
    kt = klo_max + khi_max if ant else ksub1
    nsub = nblk * kt
    cap = kt * P
    in_edge = []
    for dcore in range(CORES):
        sel, dloc, blk, cnt = per_dev[dcore]
        srcx = np.zeros((nblk, cap), dtype=np.int32)
        dl = np.full((nblk, cap), 255.0, dtype=np.float32)
        eav = np.zeros((nblk, cap), dtype=np.float32)
        starts = np.concatenate([[0], np.cumsum(cnt)])
        for b in range(nblk):
            s0, s1 = starts[b], starts[b + 1]
            seg = sel[s0:s1]
            if ant:
                # lo edges fill subtiles [0, klo_max), hi the rest
                slo = seg[lo_edge[seg]]
                shi = seg[~lo_edge[seg]]
                for part, base in ((slo, 0), (shi, klo_max * P)):
                    so = np.argsort(src_p[part], kind="stable")
                    part = part[so]
                    m = len(part)
                    srcx[b, base:base + m] = src_p[part]
                    dl[b, base:base + m] = (dst_p[part] - dcore * npad) - b * P
                    eav[b, base:base + m] = ea[part]
            else:
                m = s1 - s0
                # sort the block's edges by source row for gather locality
                so = np.argsort(src_p[seg], kind="stable")
                seg = seg[so]
                srcx[b, :m] = src_p[seg]
                dl[b, :m] = (dst_p[seg] - dcore * npad) - b * P
                eav[b, :m] = ea[seg]
        # [nblk, cap] -> [P, nsub]: subtile k of block b at col b*kt+k,
        # edge slot p on partition p
        def to_tiles(a):
            return np.ascontiguousarray(
                a.reshape(nblk, kt, P).transpose(2, 0, 1).reshape(P, nsub))
        # wrapped int16 indices for dma_gather: per block, gather order
        # i = k*128 + p, stored at [i % 16, i // 16], 16-row pattern
        # replicated across all 128 partitions; hi indices offset by -split
        sx = srcx.reshape(nblk, kt * P).astype(np.int64)
        if ant:
            sx = sx - (sx >= split) * split
        gw = sx.reshape(nblk, kt * P // 16, 16).transpose(2, 0, 1)
        gidx = np.ascontiguousarray(
            np.tile(gw, (8, 1, 1)).reshape(P, nsub * 8)).astype(np.int16)
        in_edge.append({
            "src_idx": to_tiles(srcx),
            "gidx": gidx,
            "dst_loc": to_tiles(dl),
            "ea": to_tiles(eav),
        })
    return nloc, npad, ksub, in_edge, pos, order


def make_in_maps(inputs):
    x = np.asarray(inputs["x"], dtype=np.float32)
    nloc, npad, ksub, in_edge, pos, order = preprocess(
        x, inputs["edge_index"], inputs["edge_attr"])

    def f32(name):
        return np.asarray(inputs[name], dtype=np.float32)

    w_gat = f32("gat_W")
    att_srcT = np.ascontiguousarray(f32("att_src").transpose(0, 2, 1))
    att_dstT = np.ascontiguousarray(f32("att_dst").transpose(0, 2, 1))
    edge_WT = np.ascontiguousarray(
        f32("edge_W").reshape(DEPTH, H, C).transpose(0, 2, 1))
    att_edgeT = np.ascontiguousarray(f32("att_edge").transpose(0, 2, 1))
    gat_biasT = np.ascontiguousarray(
        f32("gat_bias").reshape(DEPTH, H, C).transpose(0, 2, 1))
    ff_b1r = np.ascontiguousarray(f32("ff_b1").reshape(DEPTH, 1, MLP))


    shared = {
        "gat_W": w_gat,
        "att_srcT": att_srcT, "att_dstT": att_dstT,
        "edge_WT": edge_WT, "att_edgeT": att_edgeT, "gat_biasT": gat_biasT,
        "qf_W": f32("qf_W"), "qf_b": f32("qf_b"),
        "ln1_gT": f32("ln1_g")[:, :, None], "ln1_bT": f32("ln1_b")[:, :, None],
        "ln2_gT": f32("ln2_g")[:, :, None], "ln2_bT": f32("ln2_b")[:, :, None],
        "ff_W1": f32("ff_W1"), "ff_b1r": ff_b1r,
        "ff_W2": f32("ff_W2"), "ff_b2": f32("ff_b2"),
    }
    in_maps = []
    for dcore in range(CORES):
        xs = x[dcore * nloc:(dcore + 1) * nloc]
        xp = np.zeros((npad, D), np.float32)
        valid = order[dcore] >= 0
        xp[valid] = xs[order[dcore][valid]]
        m = {"x": xp, **in_edge[dcore], **shared}
        in_maps.append(m)
    return nloc, npad, ksub, in_maps, pos


# ----------------------------------------------------------------------------
# PJRT runner (build once, reuse executable)
# ----------------------------------------------------------------------------

_CACHE = {}


def _make_runner(nc, n_cores):
    import hashlib
    import os
    import time
    import jax
    import jax.numpy as jnp
    from jax.sharding import Mesh, PartitionSpec, NamedSharding
    from jax.experimental.shard_map import shard_map
    from concourse.bass2jax import _bass_exec_p, partition_id_tensor

    # The PJRT-level MODULE hash that keys the neuronxcc NEFF cache does not
    # cover the bass program carried in the custom-call backend_config, so two
    # different kernels can collide on the same cached NEFF. Namespace the
    # cache by a digest of the BIR to make it content-sensitive.
    bir_digest = hashlib.sha1(nc.to_json_bytes()).hexdigest()[:20]
    cache_url = f"/root/.neuron-compile-cache-bass/{bir_digest}"

    def _set_cache():
        os.environ["NEURON_COMPILE_CACHE_URL"] = cache_url

    in_names, out_names, out_avals = [], [], []
    pname = nc.partition_id_tensor.name if nc.partition_id_tensor else None
    for alloc in nc.m.functions[0].allocations:
        if not isinstance(alloc, mybir.MemoryLocationSet):
            continue
        nm = alloc.memorylocations[0].name
        if alloc.kind == "ExternalInput" and nm != pname:
            in_names.append(nm)
        elif alloc.kind == "ExternalOutput":
            out_names.append(nm)
            out_avals.append(jax.core.ShapedArray(
                tuple(alloc.tensor_shape), mybir.dt.np(alloc.dtype)))
    n_params, n_outs = len(in_names), len(out_names)
    all_names = in_names + out_names + ([pname] if pname else [])
    donate = tuple(range(n_params, n_params + n_outs))

    def _body(*args):
        operands = list(args)
        if pname:
            operands.append(partition_id_tensor())
        return tuple(_bass_exec_p.bind(
            *operands, out_avals=tuple(out_avals), in_names=tuple(all_names),
            out_names=tuple(out_names), lowering_input_output_aliases=(),
            sim_require_finite=False, sim_require_nnan=False, nc=nc))

    if os.environ.get("BASS_SIM") == "1":
        devices = jax.devices("cpu")
        if len(devices) < n_cores:
            raise RuntimeError(
                f"BASS_SIM needs {n_cores} cpu devices; set "
                f"XLA_FLAGS=--xla_force_host_platform_device_count={n_cores}")
        devices = devices[:n_cores]
    else:
        devices = jax.devices()[:n_cores]
    mesh = Mesh(np.asarray(devices), ("core",))
    sharded = jax.jit(
        shard_map(_body, mesh=mesh,
                  in_specs=(PartitionSpec("core"),) * (n_params + n_outs),
                  out_specs=(PartitionSpec("core"),) * n_outs,
                  check_rep=False),
        donate_argnums=donate, keep_unused=True)
    shard = NamedSharding(mesh, PartitionSpec("core"))
    zero_shapes = [(n_cores * a.shape[0], *a.shape[1:]) for a in out_avals]
    zero_dtypes = [a.dtype for a in out_avals]
    make_zeros = jax.jit(
        lambda: tuple(jnp.zeros(s, d) for s, d in zip(zero_shapes, zero_dtypes)),
        out_shardings=tuple(shard for _ in out_avals))

    def run(in_maps, n_timing_iters=0, return_caller=False):
        concat_in = [
            jax.device_put(np.concatenate(
                [np.ascontiguousarray(m[nm]) for m in in_maps], axis=0), shard)
            for nm in in_names
        ]

        def call():
            _set_cache()
            zeros = make_zeros()
            jax.block_until_ready(zeros)
            t0 = time.perf_counter()
            out = sharded(*concat_in, *zeros)
            jax.block_until_ready(out)
            return out, time.perf_counter() - t0

        out_arrs = None
        for attempt in range(3):
            try:
                out_arrs, _ = call()
                break
            except Exception:
                if attempt == 2:
                    raise
                time.sleep(10.0)
        best = None
        for _ in range(n_timing_iters):
            out_arrs, dt = call()
            best = dt if best is None else min(best, dt)
        results = [
            {nm: np.asarray(out_arrs[i]).reshape(n_cores, *out_avals[i].shape)[c]
             for i, nm in enumerate(out_names)}
            for c in range(n_cores)
        ]
        if return_caller:
            return results, (lambda: call()[1] * 1e9)
        return results, (None if best is None else best * 1e9)

    return run


def run_kernel(inputs, n_timing_iters=0):
    nloc, npad, ksub, in_maps, pos = make_in_maps(inputs)
    key = (npad, ksub)
    if key not in _CACHE:
        nc = build_nc(npad, ksub)
        _CACHE[key] = _make_runner(nc, CORES)
    results, best_ns = _CACHE[key](in_maps, n_timing_iters=n_timing_iters)
    out = np.concatenate(
        [results[c]["x_out"][pos[c]] for c in range(CORES)], axis=0)
    return out, best_ns


def kernel(**inputs):
    out, _ = run_kernel(inputs)
    return out
